# revision 1
# baseline (speedup 1.0000x reference)
"""Trainium2 Bass kernel for nn_NexusV2 (CentroidAddressableManifold.read).

Strategy: shard by *bucket* (not token). Tokens are routed host-side to the
core owning their bucket; each bucket's 32 slot rows are loaded exactly once
from HBM (vs. the reference's per-token gather => ~8x less memory traffic).

Device layout (per core, all shapes static at trace time):
  - tokens are packed into "instances" of <=16 tokens sharing one bucket
    (buckets with >16 tokens split into several instances)
  - groups of <=8 instances => <=128 token rows x <=256 slot columns
  - per group: PE computes scores = unified_query @ K^T (token-major,
    float32r), softmax + hard-match path on DVE/ACT, val = probs @ V on PE.

Host does only routing/permutation + transposed packing of the read-only
tables; all FLOPs of the reference (norms, dots, softmax, matches, matmuls)
run on device.
"""

import math
import sys
import types

import numpy as np

N_BUCKETS = 512
SPB = 32          # slots per bucket
TAU = 0.1
P_PAD = 16        # token rows per instance
IPG = 8           # instances per (full) group
N_CORES = 8
D = 1024
KCH = 8           # D / 128 contraction chunks
NEG = -30000.0    # additive mask value

_COMPILED = {}    # plan -> (nc, names)
_HOOK_DONE = False


# ----------------------------------------------------------------- utilities

def _install_ntff_hook():
    """Synthesize antenv.axon_hooks so trace=True can NTFF-profile (optional)."""
    global _HOOK_DONE
    if _HOOK_DONE or 'antenv.axon_hooks' in sys.modules:
        _HOOK_DONE = True
        return
    try:
        import antenv
        m = types.ModuleType('antenv.axon_hooks')
        _hook = [None]
        m.set_axon_ntff_profile_hook = lambda h: _hook.__setitem__(0, h)
        m.get_axon_ntff_profile_hook = lambda: _hook[0]
        sys.modules['antenv.axon_hooks'] = m
        antenv.axon_hooks = m
        if '/root/.axon_site' not in sys.path:
            sys.path.insert(0, '/root/.axon_site')
        from trn_agent_boot.trn_boot import _ntff_profile_via_ctypes
        m.set_axon_ntff_profile_hook(
            _ntff_profile_via_ctypes('/opt/axon/libaxon_pjrt.so'))
    except Exception:
        pass
    _HOOK_DONE = True


def _routing(tids_flat):
    """Return list of instances: (bucket_id, np.array of <=16 token indices)."""
    buckets = (tids_flat.astype(np.int64)) % N_BUCKETS
    order = np.argsort(buckets, kind='stable')
    counts = np.bincount(buckets, minlength=N_BUCKETS)
    cum = np.concatenate([[0], np.cumsum(counts)])
    instances = []
    for b in range(N_BUCKETS):
        c = int(counts[b])
        if c == 0:
            continue
        toks = order[cum[b]:cum[b] + c]
        for i in range(0, c, P_PAD):
            instances.append((b, toks[i:i + P_PAD]))
    return instances


def _plan(n_inst):
    i_core = (n_inst + N_CORES - 1) // N_CORES
    ngs, r = [], i_core
    while r > 0:
        ngs.append(min(IPG, r))
        r -= min(IPG, r)
    return i_core, tuple(ngs)


def _group_geom(ng):
    """Column geometry inside a group's kv block: KCH chunks of [K^T_k|a^T_k]
    (ns+ngp cols each), then nv V blocks of D cols. ngp = anchor dim padded
    even for fp32r matmul free-size restrictions."""
    ns = SPB * ng
    nv = 1 if ns <= 128 else 2
    ngp = ng + (ng % 2)
    return ns, nv, ngp, KCH * (ns + ngp) + nv * D


def _consts():
    r = np.arange(128)
    c256 = np.arange(256)
    valid = (c256[None, :] // SPB) == (r[:, None] // P_PAD)
    winadd = np.where(valid, 0.0, NEG).astype(np.float32)
    win01 = valid.astype(np.float32)
    oh8 = (np.arange(IPG)[None, :] == (r[:, None] // P_PAD)).astype(np.float32)
    oh8t_half = (0.5 * oh8.T).astype(np.float32)
    ident = np.eye(128, dtype=np.float32)
    return winadd, win01, oh8, oh8t_half, ident


def _pack_core(insts, ngs, q_flat, tids_flat, KT, V, slot_tids, CBT):
    """Build this core's input arrays. insts: list of (bucket, toks) or None."""
    i16 = sum(ngs) * P_PAD
    qr = np.zeros((i16, D), np.float32)
    trp = np.full((i16, 1), -1.0, np.float32)
    tidb = np.full((len(ngs), 2 * 128), -2.0, np.float32)
    tok_idx = np.full(i16, -1, np.int64)

    wtot = sum(_group_geom(ng)[3] for ng in ngs)
    kv = np.zeros((128, wtot), np.float32)

    col = 0
    row = 0
    ii = 0
    for g, ng in enumerate(ngs):
        ns, nv, ngp, wg = _group_geom(ng)
        group = insts[ii:ii + ng]
        ii += ng
        # slot ids (and bucket list) for this group
        slot_ids = np.zeros(ns, np.int64)
        real_slots = np.zeros(ns, bool)
        bucks = np.zeros(ng, np.int64)
        real_inst = np.zeros(ng, bool)
        for j, inst in enumerate(group):
            if inst is None:
                continue
            b, toks = inst
            bucks[j] = b
            real_inst[j] = True
            slot_ids[j * SPB:(j + 1) * SPB] = np.arange(b * SPB, (b + 1) * SPB)
            real_slots[j * SPB:(j + 1) * SPB] = True
            tidb[g, j * SPB:(j + 1) * SPB] = slot_tids[b * SPB:(b + 1) * SPB]
            r0 = row + j * P_PAD
            nt_real = len(toks)
            qr[r0:r0 + nt_real] = q_flat[toks]
            trp[r0:r0 + nt_real, 0] = tids_flat[toks]
            tok_idx[r0:r0 + nt_real] = toks
        # ka chunks [128, KCH, ns+ngp]: per chunk K^T slots then anchors
        ktg = KT[:, slot_ids].reshape(KCH, 128, ns) * real_slots[None, None, :]
        atp = np.zeros((KCH, 128, ngp), np.float32)
        atp[:, :, 0:ng] = CBT[:, bucks].reshape(KCH, 128, ng) \
            * real_inst[None, None, :]
        ka = np.concatenate([ktg, atp], axis=2)        # [KCH, 128, ns+ngp]
        kv[:, col:col + KCH * (ns + ngp)] = \
            ka.transpose(1, 0, 2).reshape(128, KCH * (ns + ngp))
        c = col + KCH * (ns + ngp)
        # V blocks
        vb = V[slot_ids] * real_slots[:, None]          # [ns, D]
        for h in range(nv):
            nsh = min(128, ns - h * 128)
            kv[0:nsh, c:c + D] = vb[h * 128:h * 128 + nsh]
            c += D
        col += wg
        row += ng * P_PAD
    return dict(qr=qr, kv=kv, tidb=tidb, trp=trp), tok_idx


# ------------------------------------------------------------- device kernel

def _build_nc(ngs, i16):
    from concourse import bacc, mybir, tile

    F32 = mybir.dt.float32
    F32R = mybir.dt.float32r
    AL = mybir.AluOpType
    AF = mybir.ActivationFunctionType
    X = mybir.AxisListType.X

    geoms = [_group_geom(ng) for ng in ngs]
    wtot = sum(g[3] for g in geoms)
    n_groups = len(ngs)
    kcols = np.concatenate([[0], np.cumsum([g[3] for g in geoms])])
    rows = np.concatenate([[0], np.cumsum([ng * P_PAD for ng in ngs])])

    nc = bacc.Bacc(trn_type="TRN2", target_bir_lowering=False, debug=False)
    d_qr = nc.dram_tensor("qr", [i16, D], F32, kind="ExternalInput").ap()
    d_kv = nc.dram_tensor("kv", [128, wtot], F32R, kind="ExternalInput").ap()
    d_tidb = nc.dram_tensor("tidb", [n_groups, 256], F32, kind="ExternalInput").ap()
    d_trp = nc.dram_tensor("trp", [i16, 1], F32, kind="ExternalInput").ap()
    d_winadd = nc.dram_tensor("winadd", [128, 256], F32, kind="ExternalInput").ap()
    d_win01 = nc.dram_tensor("win01", [128, 256], F32, kind="ExternalInput").ap()
    d_oh8 = nc.dram_tensor("oh8", [128, IPG], F32, kind="ExternalInput").ap()
    d_oh8t = nc.dram_tensor("oh8t", [IPG, 128], F32R, kind="ExternalInput").ap()
    d_ident = nc.dram_tensor("identw", [128, 128], F32, kind="ExternalInput").ap()
    d_out = nc.dram_tensor("outp", [i16, D], F32, kind="ExternalOutput").ap()

    with tile.TileContext(nc) as tc:
        with tc.tile_pool(name="const", bufs=1) as pc, \
             tc.tile_pool(name="kvp", bufs=4) as pkv, \
             tc.tile_pool(name="io", bufs=3) as pio, \
             tc.tile_pool(name="wk", bufs=2) as pw, \
             tc.tile_pool(name="ps", bufs=1, space="PSUM") as pp, \
             tc.tile_pool(name="ps2", bufs=2, space="PSUM") as pp2:

            winadd = pc.tile([128, 256], F32)
            win01 = pc.tile([128, 256], F32)
            oh8 = pc.tile([128, IPG], F32)
            oh8t = pc.tile([IPG, 128], F32R)
            ident = pc.tile([128, 128], F32)
            nc.sync.dma_start(winadd[:], d_winadd)
            nc.sync.dma_start(win01[:], d_win01)
            nc.sync.dma_start(oh8[:], d_oh8)
            nc.sync.dma_start(oh8t[:], d_oh8t)
            nc.sync.dma_start(ident[:], d_ident)
            eps24 = pc.tile([128, 1], F32)
            nc.gpsimd.memset(eps24[:], 1e-24)

            for g, ng in enumerate(ngs):
                ns, nv, ngp, wg = geoms[g]
                nt = ng * P_PAD
                nsp = ns + ngp
                col = kcols[g]

                kv_t = pkv.tile([128, 4160], F32R, tag="kv")
                nc.sync.dma_start(kv_t[:, 0:wg], d_kv[:, col:col + wg])
                ka = kv_t[:, 0:KCH * nsp].rearrange("p (k s) -> p k s", k=KCH)
                q_t = pio.tile([128, D], F32, tag="q")
                nc.scalar.dma_start(q_t[0:nt, :], d_qr[rows[g]:rows[g] + nt, :])
                tidb_t = pio.tile([128, 256], F32, tag="tidb")
                nc.sync.dma_start(tidb_t[0:nt, 0:ns],
                                  d_tidb[g:g + 1, 0:ns].to_broadcast((nt, ns)))
                tr_t = pio.tile([128, 1], F32, tag="tr")
                nc.scalar.dma_start(tr_t[0:nt, :], d_trp[rows[g]:rows[g] + nt, :])

                # --- normalize queries: qs = 0.5 * q / ||q||
                sq = pw.tile([128, D], F32, tag="sq")
                ssq = pw.tile([128, 1], F32, tag="ssq")
                nc.scalar.activation(sq[0:nt, :], q_t[0:nt, :], AF.Square,
                                     accum_out=ssq[0:nt, :])
                n2 = pw.tile([128, 1], F32, tag="n2")
                nc.scalar.activation(n2[0:nt, :], ssq[0:nt, :], AF.Sqrt,
                                     scale=4.0, bias=eps24[0:nt, :])
                rq2 = pw.tile([128, 1], F32, tag="rq2")
                nc.vector.reciprocal(rq2[0:nt, :], n2[0:nt, :])
                qs = pw.tile([128, D], F32, tag="qs")
                nc.vector.tensor_scalar(out=qs[0:nt, :], in0=q_t[0:nt, :],
                                        scalar1=rq2[0:nt, :], scalar2=None,
                                        op0=AL.mult)

                # --- transpose scaled queries -> qt [128d, KCH, nt] (f32r)
                qt = pw.tile([128, KCH, 128], F32R, tag="qt")
                for hb in range(2):
                    qth = pp2.tile([128, 512], F32, tag="qth")
                    for k in range(4):
                        kk = hb * 4 + k
                        nc.tensor.transpose(
                            qth[:, k * 128:k * 128 + nt],
                            qs[0:nt, kk * 128:(kk + 1) * 128],
                            ident[0:nt, 0:nt])
                    nc.vector.tensor_copy(
                        qt[:, hb * 4:(hb + 1) * 4, 0:nt],
                        qth[:].rearrange("p (k t) -> p k t", k=4)[:, :, 0:nt])

                # --- anchor-dot table a0t = a.K  [ngp, ns]
                a0t_ps = pp.tile([IPG, 256], F32, tag="a0t")
                for k in range(KCH):
                    nc.tensor.matmul(a0t_ps[0:ngp, 0:ns], ka[:, k, ns:nsp],
                                     ka[:, k, 0:ns], start=(k == 0),
                                     stop=(k == KCH - 1))
                a0t = pw.tile([IPG, 256], F32R, tag="a0tsb")
                nc.vector.tensor_copy(a0t[0:ngp, 0:ns], a0t_ps[0:ngp, 0:ns])

                # --- scores(+qa cols)+blend: [qn'.K | qn'.a] + 0.5*a.K
                sc_ps = pp.tile([128, 264], F32, tag="sc")
                for k in range(KCH):
                    nc.tensor.matmul(sc_ps[0:nt, 0:nsp], qt[:, k, 0:nt],
                                     ka[:, k, 0:nsp], start=(k == 0),
                                     stop=False)
                nc.tensor.matmul(sc_ps[0:nt, 0:ns], oh8t[0:ngp, 0:nt],
                                 a0t[0:ngp, 0:ns], start=False, stop=True)

                # --- rw = 1/|W| from qa cols; rw10 = rw/tau
                qasc = pw.tile([128, IPG], F32, tag="qasc")
                qa1 = pw.tile([128, 1], F32, tag="qa1")
                nc.vector.tensor_tensor(out=qasc[0:nt, 0:ngp],
                                        in0=sc_ps[0:nt, ns:nsp],
                                        in1=oh8[0:nt, 0:ngp], op=AL.mult)
                nc.vector.reduce_sum(qa1[0:nt, :], qasc[0:nt, 0:ngp], axis=X)
                w2 = pw.tile([128, 1], F32, tag="w2")
                nc.vector.tensor_scalar(out=w2[0:nt, :], in0=qa1[0:nt, :],
                                        scalar1=0.5, scalar2=None, op0=AL.add)
                wn = pw.tile([128, 1], F32, tag="wn")
                nc.scalar.activation(wn[0:nt, :], w2[0:nt, :], AF.Sqrt)
                rw = pw.tile([128, 1], F32, tag="rw")
                nc.vector.reciprocal(rw[0:nt, :], wn[0:nt, :])
                rw10 = pw.tile([128, 1], F32, tag="rw10")
                nc.vector.tensor_scalar(out=rw10[0:nt, :], in0=rw[0:nt, :],
                                        scalar1=1.0 / TAU, scalar2=None,
                                        op0=AL.mult)

                # --- masked scores, softmax with rw/tau in Exp scale
                sc = pw.tile([128, 256], F32, tag="scsb")
                nc.vector.tensor_tensor(out=sc[0:nt, 0:ns],
                                        in0=sc_ps[0:nt, 0:ns],
                                        in1=winadd[0:nt, 0:ns], op=AL.add)
                negmax = pw.tile([128, 1], F32, tag="negmax")
                nc.vector.reduce_max(negmax[0:nt, :], sc[0:nt, 0:ns], axis=X,
                                     negate=True)
                ebias = pw.tile([128, 1], F32, tag="ebias")
                nc.vector.tensor_tensor(out=ebias[0:nt, :], in0=negmax[0:nt, :],
                                        in1=rw10[0:nt, :], op=AL.mult)
                ex = pw.tile([128, 256], F32, tag="ex")
                esum = pw.tile([128, 1], F32, tag="esum")
                nc.scalar.activation(ex[0:nt, 0:ns], sc[0:nt, 0:ns], AF.Exp,
                                     bias=ebias[0:nt, :], scale=rw10[0:nt, :],
                                     accum_out=esum[0:nt, :])
                rsum = pw.tile([128, 1], F32, tag="rsum")
                nc.vector.reciprocal(rsum[0:nt, :], esum[0:nt, :])

                # --- hard match path
                match = pw.tile([128, 256], F32, tag="match")
                msum = pw.tile([128, 1], F32, tag="msum")
                nc.vector.scalar_tensor_tensor(
                    out=match[0:nt, 0:ns], in0=tidb_t[0:nt, 0:ns],
                    scalar=tr_t[0:nt, :], in1=win01[0:nt, 0:ns],
                    op0=AL.is_equal, op1=AL.mult, accum_out=msum[0:nt, :])
                mden = pw.tile([128, 1], F32, tag="mden")
                nc.vector.tensor_scalar(out=mden[0:nt, :], in0=msum[0:nt, :],
                                        scalar1=1e-9, scalar2=None, op0=AL.add)
                mrec = pw.tile([128, 1], F32, tag="mrec")
                nc.vector.reciprocal(mrec[0:nt, :], mden[0:nt, :])
                nohas = pw.tile([128, 1], F32, tag="nohas")
                nc.vector.tensor_scalar(out=nohas[0:nt, :], in0=msum[0:nt, :],
                                        scalar1=0.0, scalar2=None, op0=AL.is_le)
                hard = pw.tile([128, 256], F32, tag="hard")
                nc.vector.tensor_scalar(out=hard[0:nt, 0:ns],
                                        in0=match[0:nt, 0:ns],
                                        scalar1=mrec[0:nt, :], scalar2=None,
                                        op0=AL.mult)
                rs_nh = pw.tile([128, 1], F32, tag="rs_nh")
                nc.vector.tensor_tensor(out=rs_nh[0:nt, :], in0=rsum[0:nt, :],
                                        in1=nohas[0:nt, :], op=AL.mult)
                probs = pw.tile([128, 256], F32, tag="probs")
                nc.vector.scalar_tensor_tensor(
                    out=probs[0:nt, 0:ns], in0=ex[0:nt, 0:ns],
                    scalar=rs_nh[0:nt, :], in1=hard[0:nt, 0:ns],
                    op0=AL.mult, op1=AL.add)

                # --- probs^T, then val = probs @ V
                pt_ps = pp.tile([128, 264], F32, tag="pt")
                for h in range(nv):
                    nsh = min(128, ns - h * 128)
                    nc.tensor.transpose(pt_ps[0:nsh, h * 128:h * 128 + nt],
                                        probs[0:nt, h * 128:h * 128 + nsh],
                                        ident[0:nt, 0:nt])
                pt = pw.tile([128, 2, 128], F32R, tag="ptsb")
                for h in range(nv):
                    nsh = min(128, ns - h * 128)
                    nc.vector.tensor_copy(pt[0:nsh, h, 0:nt],
                                          pt_ps[0:nsh, h * 128:h * 128 + nt])
                pv = pp.tile([128, D], F32, tag="pv")
                for j in range(2):
                    for h in range(nv):
                        nsh = min(128, ns - h * 128)
                        nc.tensor.matmul(
                            pv[0:nt, j * 512:(j + 1) * 512],
                            pt[0:nsh, h, 0:nt],
                            kv_t[0:nsh, KCH * nsp + h * D + j * 512:
                                 KCH * nsp + h * D + (j + 1) * 512],
                            start=(h == 0), stop=(h == nv - 1))
                out_sb = pw.tile([128, D], F32, tag="out_sb")
                nc.vector.tensor_copy(out_sb[0:nt, :], pv[0:nt, :])
                nc.scalar.dma_start(d_out[rows[g]:rows[g] + nt, :],
                                    out_sb[0:nt, :])
    nc.compile()
    return nc


# ------------------------------------------------------------------ emulator

def _emulate_core(ins, ngs):
    """Numpy emulation of the device kernel (fp32), for validation."""
    qr, kv, tidb, trp = ins["qr"], ins["kv"], ins["tidb"], ins["trp"]
    i16 = qr.shape[0]
    out = np.zeros((i16, D), np.float32)
    winadd, win01, oh8, oh8t, _ = _consts()
    col = row = 0
    for g, ng in enumerate(ngs):
        ns, nv, ngp, wg = _group_geom(ng)
        nt = ng * P_PAD
        ka = kv[:, col:col + KCH * (ns + ngp)].reshape(128, KCH, ns + ngp)
        ktg = ka[:, :, 0:ns]
        atp = ka[:, :, ns:ns + ng]
        voff = col + KCH * (ns + ngp)
        vb = np.zeros((ns, D), np.float32)
        for h in range(nv):
            nsh = min(128, ns - h * 128)
            vb[h * 128:h * 128 + nsh] = kv[0:nsh, voff + h * D:voff + (h + 1) * D]

        q = qr[row:row + nt]
        ssq = (q * q).sum(-1, keepdims=True)
        rq2 = 1.0 / np.sqrt(4 * ssq + 1e-24)
        qn = q * rq2                                   # 0.5 * normalized
        KT = ktg.transpose(1, 0, 2).reshape(D, ns)     # [D, ns]
        AT = atp.transpose(1, 0, 2).reshape(D, ng)     # [D, ng]
        a0t = AT.T @ KT                                # [ng, ns]
        sc_ps = qn @ KT + (0.5 * oh8[0:nt, 0:ng]) @ a0t
        qa1 = ((qn @ AT) * oh8[0:nt, 0:ng]).sum(-1, keepdims=True)
        rw = 1.0 / np.sqrt(qa1 + 0.5)
        sc = sc_ps * rw + winadd[0:nt, 0:ns]
        m = sc.max(-1, keepdims=True)
        ex = np.exp((sc - m) / TAU)
        esum = ex.sum(-1, keepdims=True)
        match = (tidb[g, 0:ns][None, :] == trp[row:row + nt]) * win01[0:nt, 0:ns]
        msum = match.sum(-1, keepdims=True)
        nohas = (msum <= 0).astype(np.float32)
        hard = match / (msum + 1e-9)
        probs = ex * (nohas / esum) + hard
        out[row:row + nt] = probs.astype(np.float32) @ vb
        col += wg
        row += nt
    return out


# -------------------------------------------------------------------- kernel

def kernel(query_emb, tids, slot_keys, slot_values, slot_tids,
           centroid_codebook, _emulate=False, _trace=False):
    B, T, _ = query_emb.shape
    BT = B * T
    q_flat = np.ascontiguousarray(query_emb.reshape(BT, D), np.float32)
    tids_flat = np.asarray(tids).reshape(BT)
    st = np.asarray(slot_tids).astype(np.float32)
    KT = np.ascontiguousarray(np.asarray(slot_keys, np.float32).T)     # [D, S]
    V = np.asarray(slot_values, np.float32)
    CBT = np.ascontiguousarray(np.asarray(centroid_codebook, np.float32).T)

    instances = _routing(tids_flat)
    i_core, ngs = _plan(len(instances))
    padded = instances + [None] * (i_core * N_CORES - len(instances))
    i16 = i_core * P_PAD

    winadd, win01, oh8, oh8t, ident = _consts()
    in_maps, tok_idxs = [], []
    for c in range(N_CORES):
        ins, tok_idx = _pack_core(padded[c * i_core:(c + 1) * i_core], ngs,
                                  q_flat, tids_flat, KT, V, st, CBT)
        ins.update(winadd=winadd, win01=win01, oh8=oh8, oh8t=oh8t,
                   identw=ident)
        in_maps.append(ins)
        tok_idxs.append(tok_idx)

    out_flat = np.zeros((BT, D), np.float32)
    if _emulate:
        for c in range(N_CORES):
            o = _emulate_core(in_maps[c], ngs)
            valid = tok_idxs[c] >= 0
            out_flat[tok_idxs[c][valid]] = o[valid]
        return out_flat.reshape(B, T, D).astype(np.float32)

    _install_ntff_hook()
    from concourse import bass_utils
    key = (ngs, i16)
    if key not in _COMPILED:
        _COMPILED[key] = _build_nc(ngs, i16)
    nc = _COMPILED[key]
    res = bass_utils.run_bass_kernel_spmd(
        nc, in_maps, core_ids=list(range(N_CORES)), trace=_trace)
    for c in range(N_CORES):
        o = res.results[c]["outp"]
        valid = tok_idxs[c] >= 0
        out_flat[tok_idxs[c][valid]] = o[valid]
    out = out_flat.reshape(B, T, D).astype(np.float32)
    if _trace:
        kernel._last_exec_time_ns = res.exec_time_ns
        kernel._last_results = res
    return out



# revision 6
# speedup vs baseline: 1.0865x; 1.0865x over previous
"""Trainium2 Bass kernel for nn_NexusV2 (CentroidAddressableManifold.read).

Strategy: shard by *bucket* (not token). Tokens are routed host-side to the
core owning their bucket; each bucket's 32 slot rows are loaded exactly once
from HBM (vs. the reference's per-token gather => ~8x less memory traffic).

v2 layout (per core, all shapes static at trace time):
  - tokens are packed into "instances" of <=16 tokens sharing one bucket
  - groups of <=8 instances => <=128 token rows x <=256 slot columns
  - all PE operands packed host-side in fp16 into one kv tile per group:
    per contraction chunk k (8 chunks of 128 dims):
       [ K^T slots (ns) | anchors^T (ngp) | q^T (nt) ]
    then nv V blocks of D cols, then 2 cols holding fp32 token-ids (bitcast).
  - scores = q^T-stationary matmul streaming [K|anchors|q^T]: gives raw q.K,
    q.anchor columns, and the gram block whose diagonal is ||q||^2 -- no
    on-device transposes of q, no activation Square pass.
  - sqrt/rsqrt computed as exp/ln so every ACT op uses one table set
    (natural_log_exp_and_others) => single ACT_TABLE_LOAD for whole kernel.
  - anchor term: a0t = anchors^T.K per group (PE), blended to token rows by a
    {0,1} matmul into a separate PSUM, then combined on DVE as
    sc = blend*2||q|| + q.K which equals (q.K + 2||q|| a.K); one exp scale
    alpha = 0.25/(tau*sqrt(||q||^2 * ||W||^2)) reproduces the reference's
    normalized unified-query scores exactly.
"""

import math
import sys
import types

import numpy as np

N_BUCKETS = 512
SPB = 32          # slots per bucket
TAU = 0.1
P_PAD = 16        # token rows per instance
IPG = 8           # instances per (full) group
N_CORES = 8
D = 1024
KCH = 8           # D / 128 contraction chunks
NEG = -30000.0    # additive mask value
LN2 = math.log(2.0)
BIAS_ALPHA = math.log(0.5 / TAU)

_COMPILED = {}    # plan -> nc
_HOOK_DONE = False


# ----------------------------------------------------------------- utilities

def _install_ntff_hook():
    """Synthesize antenv.axon_hooks so trace=True can NTFF-profile (optional)."""
    global _HOOK_DONE
    if _HOOK_DONE or 'antenv.axon_hooks' in sys.modules:
        _HOOK_DONE = True
        return
    try:
        import antenv
        m = types.ModuleType('antenv.axon_hooks')
        _hook = [None]
        m.set_axon_ntff_profile_hook = lambda h: _hook.__setitem__(0, h)
        m.get_axon_ntff_profile_hook = lambda: _hook[0]
        sys.modules['antenv.axon_hooks'] = m
        antenv.axon_hooks = m
        if '/root/.axon_site' not in sys.path:
            sys.path.insert(0, '/root/.axon_site')
        from trn_agent_boot.trn_boot import _ntff_profile_via_ctypes
        m.set_axon_ntff_profile_hook(
            _ntff_profile_via_ctypes('/opt/axon/libaxon_pjrt.so'))
    except Exception:
        pass
    _HOOK_DONE = True


def _routing(tids_flat):
    """Return list of instances: (bucket_id, np.array of <=16 token indices)."""
    buckets = (tids_flat.astype(np.int64)) % N_BUCKETS
    order = np.argsort(buckets, kind='stable')
    counts = np.bincount(buckets, minlength=N_BUCKETS)
    cum = np.concatenate([[0], np.cumsum(counts)])
    instances = []
    for b in range(N_BUCKETS):
        c = int(counts[b])
        if c == 0:
            continue
        toks = order[cum[b]:cum[b] + c]
        for i in range(0, c, P_PAD):
            instances.append((b, toks[i:i + P_PAD]))
    return instances


def _plan(n_inst):
    i_core = (n_inst + N_CORES - 1) // N_CORES
    ngs, r = [], i_core
    while r > 0:
        ngs.append(min(IPG, r))
        r -= min(IPG, r)
    return i_core, tuple(ngs)


def _group_geom(ng):
    """ns slot cols, nv V blocks, ngp anchors (pad even), nt token rows,
    csp chunk width, wg total kv cols for the group."""
    ns = SPB * ng
    nv = 1 if ns <= 128 else 2
    ngp = ng + (ng % 2)
    nt = P_PAD * ng
    csp = ns + ngp + nt
    wg = KCH * csp + nv * D + 2
    return ns, nv, ngp, nt, csp, wg


def _consts():
    r = np.arange(128)
    c256 = np.arange(256)
    valid = (c256[None, :] // SPB) == (r[:, None] // P_PAD)
    winadd = np.where(valid, 0.0, NEG).astype(np.float32)
    win01 = valid.astype(np.float32)
    oh8 = (np.arange(IPG)[None, :] == (r[:, None] // P_PAD)).astype(np.float32)
    oh8t16 = np.ascontiguousarray(oh8.T).astype(np.float16)
    ident16 = np.eye(128, dtype=np.float16)
    return winadd, win01, oh8, oh8t16, ident16


def _pack_core(insts, ngs, q16, tids_flat, KT16, V16, slot_tids, CBT16):
    """Build this core's input arrays. insts: list of (bucket, toks) or None."""
    i16 = sum(_group_geom(ng)[3] for ng in ngs)
    tidb = np.full((len(ngs), 256), -2.0, np.float32)
    tok_idx = np.full(i16, -1, np.int64)

    wtot = sum(_group_geom(ng)[5] for ng in ngs)
    kv = np.zeros((128, wtot), np.float16)

    col = 0
    row = 0
    ii = 0
    for g, ng in enumerate(ngs):
        ns, nv, ngp, nt, csp, wg = _group_geom(ng)
        group = insts[ii:ii + ng]
        ii += ng
        slot_ids = np.zeros(ns, np.int64)
        real_slots = np.zeros(ns, bool)
        bucks = np.zeros(ng, np.int64)
        real_inst = np.zeros(ng, bool)
        qg = np.zeros((nt, D), np.float16)
        trp = np.full(nt, -1.0, np.float32)
        for j, inst in enumerate(group):
            if inst is None:
                continue
            b, toks = inst
            bucks[j] = b
            real_inst[j] = True
            slot_ids[j * SPB:(j + 1) * SPB] = np.arange(b * SPB, (b + 1) * SPB)
            real_slots[j * SPB:(j + 1) * SPB] = True
            tidb[g, j * SPB:(j + 1) * SPB] = slot_tids[b * SPB:(b + 1) * SPB]
            r0 = j * P_PAD
            nt_real = len(toks)
            qg[r0:r0 + nt_real] = q16[toks]
            trp[r0:r0 + nt_real] = tids_flat[toks]
            tok_idx[row + r0:row + r0 + nt_real] = toks
        # chunk block [KCH, 128, csp]: K^T slots | anchors^T | q^T
        ck = np.zeros((KCH, 128, csp), np.float16)
        ck[:, :, 0:ns] = KT16[:, slot_ids].reshape(KCH, 128, ns) \
            * real_slots[None, None, :]
        ck[:, :, ns:ns + ng] = CBT16[:, bucks].reshape(KCH, 128, ng) \
            * real_inst[None, None, :]
        ck[:, :, ns + ngp:csp] = \
            np.ascontiguousarray(qg.T).reshape(KCH, 128, nt)
        kv[:, col:col + KCH * csp] = \
            ck.transpose(1, 0, 2).reshape(128, KCH * csp)
        c = col + KCH * csp
        # V blocks
        vb = V16[slot_ids] * real_slots[:, None]          # [ns, D] fp16
        for h in range(nv):
            nsh = min(128, ns - h * 128)
            kv[0:nsh, c:c + D] = vb[h * 128:h * 128 + nsh]
            c += D
        # fp32 token-ids bitcast into 2 fp16 cols (row t = token t)
        kv[0:nt, c:c + 2] = trp.view(np.float16).reshape(nt, 2)
        col += wg
        row += nt
    return dict(kv=kv, tidb=tidb), tok_idx


# ------------------------------------------------------------- device kernel

def _build_nc(ngs):
    from concourse import bacc, mybir, tile

    F16 = mybir.dt.float16
    F32 = mybir.dt.float32
    AL = mybir.AluOpType
    AF = mybir.ActivationFunctionType
    X = mybir.AxisListType.X

    geoms = [_group_geom(ng) for ng in ngs]
    wtot = sum(g[5] for g in geoms)
    i16 = sum(g[3] for g in geoms)
    n_groups = len(ngs)
    kcols = np.concatenate([[0], np.cumsum([g[5] for g in geoms])])
    rows = np.concatenate([[0], np.cumsum([g[3] for g in geoms])])
    wmax = max(g[5] for g in geoms)

    nc = bacc.Bacc(trn_type="TRN2", target_bir_lowering=False, debug=False)
    d_kv = nc.dram_tensor("kv", [128, wtot], F16, kind="ExternalInput").ap()
    d_tidb = nc.dram_tensor("tidb", [n_groups, 256], F32, kind="ExternalInput").ap()
    d_winadd = nc.dram_tensor("winadd", [128, 256], F32, kind="ExternalInput").ap()
    d_win01 = nc.dram_tensor("win01", [128, 256], F32, kind="ExternalInput").ap()
    d_oh8 = nc.dram_tensor("oh8", [128, IPG], F32, kind="ExternalInput").ap()
    d_oh8t = nc.dram_tensor("oh8t16", [IPG, 128], F16, kind="ExternalInput").ap()
    d_ident = nc.dram_tensor("ident16", [128, 128], F16, kind="ExternalInput").ap()
    d_out = nc.dram_tensor("outp", [i16, D], F16, kind="ExternalOutput").ap()

    with tile.TileContext(nc) as tc:
        with tc.tile_pool(name="const", bufs=1) as pc, \
             tc.tile_pool(name="kvp", bufs=4) as pkv, \
             tc.tile_pool(name="io", bufs=3) as pio, \
             tc.tile_pool(name="wk", bufs=2) as pw, \
             tc.tile_pool(name="ps", bufs=1, space="PSUM") as pp:

            winadd = pc.tile([128, 256], F32)
            win01 = pc.tile([128, 256], F32)
            oh8 = pc.tile([128, IPG], F32)
            oh8t = pc.tile([IPG, 128], F16)
            ident = pc.tile([128, 128], F16)
            nc.sync.dma_start(winadd[:], d_winadd)
            nc.sync.dma_start(win01[:], d_win01)
            nc.sync.dma_start(oh8[:], d_oh8)
            nc.sync.dma_start(oh8t[:], d_oh8t)
            nc.sync.dma_start(ident[:], d_ident)
            eps12 = pc.tile([128, 1], F32)
            nc.gpsimd.memset(eps12[:], 1e-12)
            balpha = pc.tile([128, 1], F32)
            nc.gpsimd.memset(balpha[:], BIAS_ALPHA)

            for g, ng in enumerate(ngs):
                ns, nv, ngp, nt, csp, wg = geoms[g]
                col = kcols[g]

                kv_t = pkv.tile([128, wmax], F16, tag="kv")
                nc.sync.dma_start(kv_t[:, 0:wg], d_kv[:, col:col + wg])
                ka = kv_t[:, 0:KCH * csp].rearrange("p (k s) -> p k s", k=KCH)
                voff = KCH * csp
                tr_t = kv_t[:, wg - 2:wg].bitcast(F32)
                tidb_t = pio.tile([128, 256], F32, tag="tidb")
                nc.sync.dma_start(tidb_t[0:nt, 0:ns],
                                  d_tidb[g:g + 1, 0:ns].to_broadcast((nt, ns)))

                # --- raw scores q.K [nt, ns] and ext cols [q.a | gram]
                qk_ps = pp.tile([128, 256], F32, tag="qk")
                ext_ps = pp.tile([128, 136], F32, tag="ext")
                for k in range(KCH):
                    q_l = ka[:, k, ns + ngp:csp]
                    nc.tensor.matmul(qk_ps[0:nt, 0:ns], q_l, ka[:, k, 0:ns],
                                     start=(k == 0), stop=(k == KCH - 1))
                    nc.tensor.matmul(ext_ps[0:nt, 0:ngp + nt], q_l,
                                     ka[:, k, ns:csp],
                                     start=(k == 0), stop=(k == KCH - 1))

                # --- anchor-dot table a0t = a.K  [ngp, ns]
                a0t_ps = pp.tile([IPG, 256], F32, tag="a0t")
                for k in range(KCH):
                    nc.tensor.matmul(a0t_ps[0:ngp, 0:ns], ka[:, k, ns:ns + ngp],
                                     ka[:, k, 0:ns], start=(k == 0),
                                     stop=(k == KCH - 1))
                a0t16 = pw.tile([IPG, 256], F16, tag="a0t16")
                nc.vector.tensor_copy(a0t16[0:ngp, 0:ns], a0t_ps[0:ngp, 0:ns])

                # --- blend a.K to token rows: bl[t, s] = a0t[inst(t), s]
                bl_ps = pp.tile([128, 256], F32, tag="bl")
                nc.tensor.matmul(bl_ps[0:nt, 0:ns], oh8t[0:ngp, 0:nt],
                                 a0t16[0:ngp, 0:ns], start=True, stop=True)

                # --- per-token scalars from ext columns
                ssq = pw.tile([128, 1], F32, tag="ssq")
                junk = pw.tile([128, 128], F16, tag="junk")
                nc.vector.scalar_tensor_tensor(
                    out=junk[0:nt, 0:nt], in0=ext_ps[0:nt, ngp:ngp + nt],
                    scalar=1.0, in1=ident[0:nt, 0:nt],
                    op0=AL.bypass, op1=AL.mult, accum_out=ssq[0:nt, :])
                qa = pw.tile([128, 1], F32, tag="qa")
                junk8 = pw.tile([128, IPG], F16, tag="junk8")
                nc.vector.scalar_tensor_tensor(
                    out=junk8[0:nt, 0:ng], in0=ext_ps[0:nt, 0:ng],
                    scalar=1.0, in1=oh8[0:nt, 0:ng],
                    op0=AL.bypass, op1=AL.mult, accum_out=qa[0:nt, :])
                # nq = 2*sqrt(ssq) = exp(0.5*ln(ssq) + ln2)
                lssq = pw.tile([128, 1], F32, tag="lssq")
                nc.scalar.activation(lssq[0:nt, :], ssq[0:nt, :], AF.Ln,
                                     bias=eps12[0:nt, :])
                nq = pw.tile([128, 1], F32, tag="nq")
                nc.scalar.activation(nq[0:nt, :], lssq[0:nt, :], AF.Exp,
                                     scale=0.5)
                rnq = pw.tile([128, 1], F32, tag="rnq")
                nc.vector.reciprocal(rnq[0:nt, :], nq[0:nt, :])
                # w2b = 0.5*qa/nq + 0.5 ; P = w2b*ssq
                w2 = pw.tile([128, 1], F32, tag="w2")
                nc.vector.tensor_scalar(out=w2[0:nt, :], in0=qa[0:nt, :],
                                        scalar1=rnq[0:nt, :], scalar2=0.5,
                                        op0=AL.mult, op1=AL.mult)
                w2b = pw.tile([128, 1], F32, tag="w2b")
                nc.vector.tensor_scalar(out=w2b[0:nt, :], in0=w2[0:nt, :],
                                        scalar1=0.5, scalar2=None, op0=AL.add)
                pprod = pw.tile([128, 1], F32, tag="pprod")
                nc.vector.tensor_tensor(out=pprod[0:nt, :], in0=w2b[0:nt, :],
                                        in1=ssq[0:nt, :], op=AL.mult)
                # alpha = 0.25/(tau*sqrt(P)) = exp(-0.5*ln(P) + ln(0.25/tau))
                lp = pw.tile([128, 1], F32, tag="lp")
                nc.scalar.activation(lp[0:nt, :], pprod[0:nt, :], AF.Ln,
                                     bias=eps12[0:nt, :])
                alpha = pw.tile([128, 1], F32, tag="alpha")
                nc.scalar.activation(alpha[0:nt, :], lp[0:nt, :], AF.Exp,
                                     scale=-0.5, bias=balpha[0:nt, :])

                # --- combine: sc = (qk + winadd) + nq*bl
                tmp = pw.tile([128, 256], F32, tag="tmp")
                nc.vector.tensor_tensor(out=tmp[0:nt, 0:ns],
                                        in0=qk_ps[0:nt, 0:ns],
                                        in1=winadd[0:nt, 0:ns], op=AL.add)
                sc = pw.tile([128, 256], F32, tag="sc")
                nc.vector.scalar_tensor_tensor(
                    out=sc[0:nt, 0:ns], in0=bl_ps[0:nt, 0:ns],
                    scalar=nq[0:nt, :], in1=tmp[0:nt, 0:ns],
                    op0=AL.mult, op1=AL.add)

                # --- softmax with alpha scale
                negmax = pw.tile([128, 1], F32, tag="negmax")
                nc.vector.reduce_max(negmax[0:nt, :], sc[0:nt, 0:ns], axis=X,
                                     negate=True)
                ebias = pw.tile([128, 1], F32, tag="ebias")
                nc.vector.tensor_tensor(out=ebias[0:nt, :], in0=negmax[0:nt, :],
                                        in1=alpha[0:nt, :], op=AL.mult)
                ex = pw.tile([128, 256], F16, tag="ex")
                esum = pw.tile([128, 1], F32, tag="esum")
                nc.scalar.activation(ex[0:nt, 0:ns], sc[0:nt, 0:ns], AF.Exp,
                                     bias=ebias[0:nt, :], scale=alpha[0:nt, :],
                                     accum_out=esum[0:nt, :])
                rsum = pw.tile([128, 1], F32, tag="rsum")
                nc.vector.reciprocal(rsum[0:nt, :], esum[0:nt, :])

                # --- hard match path
                match = pw.tile([128, 256], F32, tag="match")
                msum = pw.tile([128, 1], F32, tag="msum")
                nc.vector.scalar_tensor_tensor(
                    out=match[0:nt, 0:ns], in0=tidb_t[0:nt, 0:ns],
                    scalar=tr_t[0:nt, :], in1=win01[0:nt, 0:ns],
                    op0=AL.is_equal, op1=AL.mult, accum_out=msum[0:nt, :])
                mden = pw.tile([128, 1], F32, tag="mden")
                nc.vector.tensor_scalar(out=mden[0:nt, :], in0=msum[0:nt, :],
                                        scalar1=1e-9, scalar2=None, op0=AL.add)
                mrec = pw.tile([128, 1], F32, tag="mrec")
                nc.vector.reciprocal(mrec[0:nt, :], mden[0:nt, :])
                nohas = pw.tile([128, 1], F32, tag="nohas")
                nc.vector.tensor_scalar(out=nohas[0:nt, :], in0=msum[0:nt, :],
                                        scalar1=0.0, scalar2=None, op0=AL.is_le)
                rs_nh = pw.tile([128, 1], F32, tag="rs_nh")
                nc.vector.tensor_tensor(out=rs_nh[0:nt, :], in0=rsum[0:nt, :],
                                        in1=nohas[0:nt, :], op=AL.mult)
                hard = pw.tile([128, 256], F16, tag="hard")
                nc.vector.tensor_scalar(out=hard[0:nt, 0:ns],
                                        in0=match[0:nt, 0:ns],
                                        scalar1=mrec[0:nt, :], scalar2=None,
                                        op0=AL.mult)
                probs = pw.tile([128, 256], F16, tag="probs")
                nc.vector.scalar_tensor_tensor(
                    out=probs[0:nt, 0:ns], in0=ex[0:nt, 0:ns],
                    scalar=rs_nh[0:nt, :], in1=hard[0:nt, 0:ns],
                    op0=AL.mult, op1=AL.add)

                # --- probs^T (fp16), then val = probs @ V
                pt_ps = pp.tile([128, 264], F16, tag="pt")
                for h in range(nv):
                    nsh = min(128, ns - h * 128)
                    nc.tensor.transpose(pt_ps[0:nsh, h * 128:h * 128 + nt],
                                        probs[0:nt, h * 128:h * 128 + nsh],
                                        ident[0:nt, 0:nt])
                pt16 = pw.tile([128, 2, 128], F16, tag="pt16")
                for h in range(nv):
                    nsh = min(128, ns - h * 128)
                    nc.vector.tensor_copy(pt16[0:nsh, h, 0:nt],
                                          pt_ps[0:nsh, h * 128:h * 128 + nt])
                pv = pp.tile([128, D], F32, tag="pv")
                for j in range(2):
                    for h in range(nv):
                        nsh = min(128, ns - h * 128)
                        nc.tensor.matmul(
                            pv[0:nt, j * 512:(j + 1) * 512],
                            pt16[0:nsh, h, 0:nt],
                            kv_t[0:nsh, voff + h * D + j * 512:
                                 voff + h * D + (j + 1) * 512],
                            start=(h == 0), stop=(h == nv - 1))
                out16 = pw.tile([128, D], F16, tag="out16")
                nc.vector.tensor_copy(out16[0:nt, :], pv[0:nt, :])
                nc.scalar.dma_start(d_out[rows[g]:rows[g] + nt, :],
                                    out16[0:nt, :])
    nc.compile()
    return nc


# ------------------------------------------------------------------ emulator

def _emulate_core(ins, ngs):
    """Numpy emulation of the device kernel, for validation."""
    kv, tidb = ins["kv"], ins["tidb"]
    i16 = sum(_group_geom(ng)[3] for ng in ngs)
    out = np.zeros((i16, D), np.float32)
    winadd, win01, oh8, oh8t16, _ = _consts()
    col = row = 0
    for g, ng in enumerate(ngs):
        ns, nv, ngp, nt, csp, wg = _group_geom(ng)
        ck = kv[:, col:col + KCH * csp].reshape(128, KCH, csp)
        KT = ck[:, :, 0:ns].astype(np.float32)
        AT = ck[:, :, ns:ns + ngp].astype(np.float32)
        QT = ck[:, :, ns + ngp:csp].astype(np.float32)
        voff = col + KCH * csp
        vb = np.zeros((ns, D), np.float32)
        for h in range(nv):
            nsh = min(128, ns - h * 128)
            vb[h * 128:h * 128 + nsh] = \
                kv[0:nsh, voff + h * D:voff + (h + 1) * D].astype(np.float32)
        trp = np.ascontiguousarray(
            kv[0:nt, voff + nv * D:voff + nv * D + 2]).view(np.float32)

        KTm = KT.transpose(1, 0, 2).reshape(D, ns)
        ATm = AT.transpose(1, 0, 2).reshape(D, ngp)
        QTm = QT.transpose(1, 0, 2).reshape(D, nt)
        qk = QTm.T @ KTm                                # [nt, ns]
        exta = QTm.T @ ATm                              # [nt, ngp]
        ssq = (QTm * QTm).sum(0)[:, None]               # [nt, 1]
        a0t = ATm.T @ KTm                               # [ngp, ns]
        bl = oh8t16[0:ngp, 0:nt].astype(np.float32).T @ a0t
        qa = (exta[:, 0:ng] * oh8[0:nt, 0:ng]).sum(-1, keepdims=True)
        nq = np.exp(0.5 * np.log(ssq + 1e-12))
        w2b = 0.5 * qa / nq + 0.5
        pprod = w2b * ssq
        alpha = np.exp(-0.5 * np.log(pprod + 1e-12) + BIAS_ALPHA)
        sc = qk + winadd[0:nt, 0:ns] + nq * bl
        m = sc.max(-1, keepdims=True)
        ex = np.exp(alpha * (sc - m))
        esum = ex.sum(-1, keepdims=True)
        match = (tidb[g, 0:ns][None, :] == trp) * win01[0:nt, 0:ns]
        msum = match.sum(-1, keepdims=True)
        nohas = (msum <= 0).astype(np.float32)
        hard = match / (msum + 1e-9)
        probs = ex * (nohas / esum) + hard
        out[row:row + nt] = probs.astype(np.float16).astype(np.float32) @ vb
        col += wg
        row += nt
    return out


# -------------------------------------------------------------------- kernel

def kernel(query_emb, tids, slot_keys, slot_values, slot_tids,
           centroid_codebook, _emulate=False, _trace=False):
    B, T, _ = query_emb.shape
    BT = B * T
    q16 = np.asarray(query_emb, np.float32).reshape(BT, D).astype(np.float16)
    tids_flat = np.asarray(tids).reshape(BT)
    st = np.asarray(slot_tids).astype(np.float32)
    KT16 = np.ascontiguousarray(
        np.asarray(slot_keys, np.float32).T.astype(np.float16))   # [D, S]
    V16 = np.asarray(slot_values, np.float32).astype(np.float16)  # [S, D]
    CBT16 = np.ascontiguousarray(
        np.asarray(centroid_codebook, np.float32).T.astype(np.float16))

    instances = _routing(tids_flat)
    i_core, ngs = _plan(len(instances))
    padded = instances + [None] * (i_core * N_CORES - len(instances))

    winadd, win01, oh8, oh8t16, ident16 = _consts()
    in_maps, tok_idxs = [], []
    for c in range(N_CORES):
        ins, tok_idx = _pack_core(padded[c * i_core:(c + 1) * i_core], ngs,
                                  q16, tids_flat, KT16, V16, st, CBT16)
        ins.update(winadd=winadd, win01=win01, oh8=oh8, oh8t16=oh8t16,
                   ident16=ident16)
        in_maps.append(ins)
        tok_idxs.append(tok_idx)

    out_flat = np.zeros((BT, D), np.float32)
    if _emulate:
        for c in range(N_CORES):
            o = _emulate_core(in_maps[c], ngs)
            valid = tok_idxs[c] >= 0
            out_flat[tok_idxs[c][valid]] = o[valid]
        return out_flat.reshape(B, T, D).astype(np.float32)

    _install_ntff_hook()
    from concourse import bass_utils
    key = ngs
    if key not in _COMPILED:
        _COMPILED[key] = _build_nc(ngs)
    nc = _COMPILED[key]
    res = bass_utils.run_bass_kernel_spmd(
        nc, in_maps, core_ids=list(range(N_CORES)), trace=_trace)
    for c in range(N_CORES):
        o = np.asarray(res.results[c]["outp"], np.float32)
        valid = tok_idxs[c] >= 0
        out_flat[tok_idxs[c][valid]] = o[valid]
    out = out_flat.reshape(B, T, D).astype(np.float32)
    if _trace:
        kernel._last_exec_time_ns = res.exec_time_ns
        kernel._last_results = res
    return out


# revision 10
# speedup vs baseline: 1.4176x; 1.3047x over previous
"""Trainium2 Bass kernel for nn_NexusV2 (CentroidAddressableManifold.read).

Strategy: shard by *bucket* (not token). Tokens are routed host-side to the
core owning their bucket; each bucket's 32 slot rows are loaded exactly once
from HBM (vs. the reference's per-token gather => ~8x less memory traffic).

v2 layout (per core, all shapes static at trace time):
  - tokens are packed into "instances" of <=16 tokens sharing one bucket
  - groups of <=8 instances => <=128 token rows x <=256 slot columns
  - all PE operands packed host-side in fp16 into one kv tile per group:
    per contraction chunk k (8 chunks of 128 dims):
       [ K^T slots (ns) | anchors^T (ngp) | q^T (nt) ]
    then nv V blocks of D cols, then 2 cols holding fp32 token-ids (bitcast).
  - scores = q^T-stationary matmul streaming [K|anchors|q^T]: gives raw q.K,
    q.anchor columns, and the gram block whose diagonal is ||q||^2 -- no
    on-device transposes of q, no activation Square pass.
  - sqrt/rsqrt computed as exp/ln so every ACT op uses one table set
    (natural_log_exp_and_others) => single ACT_TABLE_LOAD for whole kernel.
  - anchor term: a0t = anchors^T.K per group (PE), blended to token rows by a
    {0,1} matmul into a separate PSUM, then combined on DVE as
    sc = blend*2||q|| + q.K which equals (q.K + 2||q|| a.K); one exp scale
    alpha = 0.25/(tau*sqrt(||q||^2 * ||W||^2)) reproduces the reference's
    normalized unified-query scores exactly.
"""

import math
import sys
import types

import numpy as np

N_BUCKETS = 512
SPB = 32          # slots per bucket
TAU = 0.1
P_PAD = 16        # token rows per instance
IPG = 8           # instances per (full) group
N_CORES = 8
D = 1024
KCH = 8           # D / 128 contraction chunks
NEG = -30000.0    # additive mask value
LN2 = math.log(2.0)
BIAS_ALPHA = math.log(0.5 / TAU)

_COMPILED = {}    # plan -> nc
_HOOK_DONE = False


# ----------------------------------------------------------------- utilities

def _install_ntff_hook():
    """Synthesize antenv.axon_hooks so trace=True can NTFF-profile (optional)."""
    global _HOOK_DONE
    if _HOOK_DONE or 'antenv.axon_hooks' in sys.modules:
        _HOOK_DONE = True
        return
    try:
        import antenv
        m = types.ModuleType('antenv.axon_hooks')
        _hook = [None]
        m.set_axon_ntff_profile_hook = lambda h: _hook.__setitem__(0, h)
        m.get_axon_ntff_profile_hook = lambda: _hook[0]
        sys.modules['antenv.axon_hooks'] = m
        antenv.axon_hooks = m
        if '/root/.axon_site' not in sys.path:
            sys.path.insert(0, '/root/.axon_site')
        from trn_agent_boot.trn_boot import _ntff_profile_via_ctypes
        m.set_axon_ntff_profile_hook(
            _ntff_profile_via_ctypes('/opt/axon/libaxon_pjrt.so'))
    except Exception:
        pass
    _HOOK_DONE = True


def _routing(tids_flat):
    """Return list of instances: (bucket_id, np.array of <=16 token indices)."""
    buckets = (tids_flat.astype(np.int64)) % N_BUCKETS
    order = np.argsort(buckets, kind='stable')
    counts = np.bincount(buckets, minlength=N_BUCKETS)
    cum = np.concatenate([[0], np.cumsum(counts)])
    instances = []
    for b in range(N_BUCKETS):
        c = int(counts[b])
        if c == 0:
            continue
        toks = order[cum[b]:cum[b] + c]
        for i in range(0, c, P_PAD):
            instances.append((b, toks[i:i + P_PAD]))
    return instances


def _plan(n_inst):
    i_core = (n_inst + N_CORES - 1) // N_CORES
    ngs, r = [], i_core
    while r > 0:
        ngs.append(min(IPG, r))
        r -= min(IPG, r)
    return i_core, tuple(ngs)


def _group_geom(ng):
    """ns slot cols, nv V blocks, ngp anchors (pad even), nt token rows,
    csp chunk width, wg total kv cols for the group."""
    ns = SPB * ng
    nv = 1 if ns <= 128 else 2
    ngp = ng + (ng % 2)
    nt = P_PAD * ng
    csp = ns + ngp + nt
    wg = KCH * csp + nv * D + 2 + 512
    return ns, nv, ngp, nt, csp, wg


def _consts():
    r = np.arange(128)
    c256 = np.arange(256)
    valid = (c256[None, :] // SPB) == (r[:, None] // P_PAD)
    winadd = np.where(valid, 0.0, NEG).astype(np.float32)
    win01 = valid.astype(np.float32)
    oh8 = (np.arange(IPG)[None, :] == (r[:, None] // P_PAD)).astype(np.float32)
    oh8t16 = np.ascontiguousarray(oh8.T).astype(np.float16)
    ident16 = np.eye(128, dtype=np.float16)
    return winadd, win01, oh8, oh8t16, ident16


def _pack_core(insts, ngs, q16, tids_flat, KT16, V16, slot_tids, CBT16):
    """Build this core's input arrays. insts: list of (bucket, toks) or None."""
    i16 = sum(_group_geom(ng)[3] for ng in ngs)
    tok_idx = np.full(i16, -1, np.int64)

    wtot = sum(_group_geom(ng)[5] for ng in ngs)
    kv = np.zeros((128, wtot), np.float16)

    col = 0
    row = 0
    ii = 0
    for g, ng in enumerate(ngs):
        ns, nv, ngp, nt, csp, wg = _group_geom(ng)
        group = insts[ii:ii + ng]
        ii += ng
        slot_ids = np.zeros(ns, np.int64)
        real_slots = np.zeros(ns, bool)
        bucks = np.zeros(ng, np.int64)
        real_inst = np.zeros(ng, bool)
        qg = np.zeros((nt, D), np.float16)
        trp = np.full(nt, -1.0, np.float32)
        tidb = np.full(256, -2.0, np.float32)
        for j, inst in enumerate(group):
            if inst is None:
                continue
            b, toks = inst
            bucks[j] = b
            real_inst[j] = True
            slot_ids[j * SPB:(j + 1) * SPB] = np.arange(b * SPB, (b + 1) * SPB)
            real_slots[j * SPB:(j + 1) * SPB] = True
            tidb[j * SPB:(j + 1) * SPB] = slot_tids[b * SPB:(b + 1) * SPB]
            r0 = j * P_PAD
            nt_real = len(toks)
            qg[r0:r0 + nt_real] = q16[toks]
            trp[r0:r0 + nt_real] = tids_flat[toks]
            tok_idx[row + r0:row + r0 + nt_real] = toks
        # chunk block [KCH, 128, csp]: K^T slots | anchors^T | q^T
        ck = np.zeros((KCH, 128, csp), np.float16)
        ck[:, :, 0:ns] = KT16[:, slot_ids].reshape(KCH, 128, ns) \
            * real_slots[None, None, :]
        ck[:, :, ns:ns + ng] = CBT16[:, bucks].reshape(KCH, 128, ng) \
            * real_inst[None, None, :]
        ck[:, :, ns + ngp:csp] = \
            np.ascontiguousarray(qg.T).reshape(KCH, 128, nt)
        kv[:, col:col + KCH * csp] = \
            ck.transpose(1, 0, 2).reshape(128, KCH * csp)
        c = col + KCH * csp
        # V blocks
        vb = V16[slot_ids] * real_slots[:, None]          # [ns, D] fp16
        for h in range(nv):
            nsh = min(128, ns - h * 128)
            kv[0:nsh, c:c + D] = vb[h * 128:h * 128 + nsh]
            c += D
        # fp32 token-ids bitcast into 2 fp16 cols (row t = token t)
        kv[0:nt, c:c + 2] = trp.view(np.float16).reshape(nt, 2)
        # fp32 slot-tid row bitcast into 512 fp16 cols on partition 0
        kv[0, c + 2:c + 2 + 512] = tidb.view(np.float16)
        col += wg
        row += nt
    return dict(kv=kv), tok_idx


# ------------------------------------------------------------- device kernel

def _build_nc(ngs):
    from concourse import bacc, mybir, tile

    F16 = mybir.dt.float16
    F32 = mybir.dt.float32
    I32 = mybir.dt.int32
    AL = mybir.AluOpType
    AF = mybir.ActivationFunctionType
    X = mybir.AxisListType.X

    geoms = [_group_geom(ng) for ng in ngs]
    wtot = sum(g[5] for g in geoms)
    i16 = sum(g[3] for g in geoms)
    n_groups = len(ngs)
    kcols = np.concatenate([[0], np.cumsum([g[5] for g in geoms])])
    rows = np.concatenate([[0], np.cumsum([g[3] for g in geoms])])
    wmax = max(g[5] for g in geoms)

    nc = bacc.Bacc(trn_type="TRN2", target_bir_lowering=False, debug=False)
    d_kv = nc.dram_tensor("kv", [128, wtot], F16, kind="ExternalInput").ap()
    d_winadd = nc.dram_tensor("winadd", [128, 256], F32, kind="ExternalInput").ap()
    d_win01 = nc.dram_tensor("win01", [128, 256], F32, kind="ExternalInput").ap()
    d_oh8 = nc.dram_tensor("oh8", [128, IPG], F32, kind="ExternalInput").ap()
    d_oh8t = nc.dram_tensor("oh8t16", [IPG, 128], F16, kind="ExternalInput").ap()
    d_ident = nc.dram_tensor("ident16", [128, 128], F16, kind="ExternalInput").ap()
    d_out = nc.dram_tensor("outp", [i16, D], F16, kind="ExternalOutput").ap()

    with tile.TileContext(nc) as tc:
        with tc.tile_pool(name="const", bufs=1) as pc, \
             tc.tile_pool(name="kvp", bufs=4) as pkv, \
             tc.tile_pool(name="io", bufs=3) as pio, \
             tc.tile_pool(name="wk", bufs=2) as pw, \
             tc.tile_pool(name="ps", bufs=1, space="PSUM") as pp:

            winadd = pc.tile([128, 256], F32)
            win01 = pc.tile([128, 256], F32)
            oh8 = pc.tile([128, IPG], F32)
            oh8t = pc.tile([IPG, 128], F16)
            ident = pc.tile([128, 128], F16)
            nc.sync.dma_start(winadd[:], d_winadd)
            nc.sync.dma_start(win01[:], d_win01)
            nc.sync.dma_start(oh8[:], d_oh8)
            nc.sync.dma_start(oh8t[:], d_oh8t)
            nc.sync.dma_start(ident[:], d_ident)
            magic = pc.tile([128, 1], I32)
            nc.gpsimd.memset(magic[:], 0x5F3759DF)
            half = pc.tile([128, 1], F32)
            nc.gpsimd.memset(half[:], 0.5)

            for g, ng in enumerate(ngs):
                ns, nv, ngp, nt, csp, wg = geoms[g]
                col = kcols[g]

                kv_t = pkv.tile([128, wmax], F16, tag="kv")
                nc.sync.dma_start(kv_t[:, 0:wg], d_kv[:, col:col + wg])
                ka = kv_t[:, 0:KCH * csp].rearrange("p (k s) -> p k s", k=KCH)
                voff = KCH * csp
                tr_t = kv_t[:, wg - 514:wg - 512].bitcast(F32)
                tidb_t = pio.tile([128, 256], F32, tag="tidb")
                nc.gpsimd.partition_broadcast(
                    tidb_t[0:nt, 0:ns],
                    kv_t[0:1, wg - 512:wg].bitcast(F32)[:, 0:ns],
                    channels=nt)

                # --- merged scores [q.K | q.a | gram] in one PSUM tile
                qke_ps = pp.tile([128, 392], F32, tag="qke", bufs=2)
                for k in range(KCH):
                    nc.tensor.matmul(qke_ps[0:nt, 0:csp],
                                     ka[:, k, ns + ngp:csp], ka[:, k, 0:csp],
                                     start=(k == 0), stop=(k == KCH - 1))

                # --- anchor-dot table a0t = a.K  [ngp, ns]
                a0t_ps = pp.tile([IPG, 256], F32, tag="a0t", bufs=2)
                for k in range(KCH):
                    nc.tensor.matmul(a0t_ps[0:ngp, 0:ns], ka[:, k, ns:ns + ngp],
                                     ka[:, k, 0:ns], start=(k == 0),
                                     stop=(k == KCH - 1))
                a0t16 = pw.tile([IPG, 256], F16, tag="a0t16")
                nc.vector.tensor_copy(a0t16[0:ngp, 0:ns], a0t_ps[0:ngp, 0:ns])

                # --- blend a.K to token rows: bl[t, s] = a0t[inst(t), s]
                bl_ps = pp.tile([128, 256], F32, tag="bl")
                nc.tensor.matmul(bl_ps[0:nt, 0:ns], oh8t[0:ngp, 0:nt],
                                 a0t16[0:ngp, 0:ns], start=True, stop=True)

                # --- per-token scalars from ext columns
                ssq = pw.tile([128, 1], F32, tag="ssq")
                junk = pw.tile([128, 128], F16, tag="junk")
                nc.vector.scalar_tensor_tensor(
                    out=junk[0:nt, 0:nt],
                    in0=qke_ps[0:nt, ns + ngp:ns + ngp + nt],
                    scalar=1.0, in1=ident[0:nt, 0:nt],
                    op0=AL.bypass, op1=AL.mult, accum_out=ssq[0:nt, :])
                qa = pw.tile([128, 1], F32, tag="qa")
                junk8 = pw.tile([128, IPG], F16, tag="junk8")
                nc.vector.scalar_tensor_tensor(
                    out=junk8[0:nt, 0:ng], in0=qke_ps[0:nt, ns:ns + ng],
                    scalar=1.0, in1=oh8[0:nt, 0:ng],
                    op0=AL.bypass, op1=AL.mult, accum_out=qa[0:nt, :])
                # quake rsqrt on DVE: no ACT table switches.
                def _rsqrt(xap, n_newton, tagp, final_scale=None,
                           final_scale_ap=None):
                    yt = pw.tile([128, 1], F32, tag=tagp + "y")
                    xs = pw.tile([128, 1], I32, tag=tagp + "xs")
                    nc.vector.tensor_scalar(
                        out=xs[0:nt, :], in0=xap.bitcast(I32), scalar1=1,
                        scalar2=None, op0=AL.logical_shift_right)
                    nc.vector.tensor_tensor(
                        out=yt[0:nt, :].bitcast(I32), in0=magic[0:nt, :],
                        in1=xs[0:nt, :], op=AL.subtract)
                    for it in range(n_newton):
                        t2 = pw.tile([128, 1], F32, tag=f"{tagp}t2{it}")
                        nc.vector.scalar_tensor_tensor(
                            out=t2[0:nt, :], in0=yt[0:nt, :],
                            scalar=xap, in1=yt[0:nt, :],
                            op0=AL.mult, op1=AL.mult)
                        t3 = pw.tile([128, 1], F32, tag=f"{tagp}t3{it}")
                        nc.vector.tensor_scalar(out=t3[0:nt, :],
                                                in0=t2[0:nt, :], scalar1=-0.5,
                                                scalar2=1.5, op0=AL.mult,
                                                op1=AL.add)
                        yo = pw.tile([128, 1], F32, tag=f"{tagp}y{it}")
                        last = it == n_newton - 1
                        if last and final_scale is not None:
                            nc.vector.tensor_scalar(
                                out=yo[0:nt, :], in0=yt[0:nt, :],
                                scalar1=t3[0:nt, :], scalar2=final_scale,
                                op0=AL.mult, op1=AL.mult)
                        elif last and final_scale_ap is not None:
                            nc.vector.scalar_tensor_tensor(
                                out=yo[0:nt, :], in0=yt[0:nt, :],
                                scalar=t3[0:nt, :], in1=final_scale_ap,
                                op0=AL.mult, op1=AL.mult)
                        else:
                            nc.vector.tensor_tensor(out=yo[0:nt, :],
                                                    in0=yt[0:nt, :],
                                                    in1=t3[0:nt, :],
                                                    op=AL.mult)
                        yt = yo
                    return yt

                ssqg = pw.tile([128, 1], F32, tag="ssqg")
                nc.vector.tensor_scalar(out=ssqg[0:nt, :], in0=ssq[0:nt, :],
                                        scalar1=1e-12, scalar2=None,
                                        op0=AL.add)
                r1 = _rsqrt(ssqg[0:nt, :], 2, "r1")          # 1/sqrt(ssq)
                nq = pw.tile([128, 1], F32, tag="nq")
                nc.vector.tensor_tensor(out=nq[0:nt, :], in0=ssqg[0:nt, :],
                                        in1=r1[0:nt, :], op=AL.mult)
                r1h = pw.tile([128, 1], F32, tag="r1h")
                nc.vector.tensor_scalar(out=r1h[0:nt, :], in0=r1[0:nt, :],
                                        scalar1=0.5, scalar2=None, op0=AL.mult)
                w2b = pw.tile([128, 1], F32, tag="w2b")
                nc.vector.scalar_tensor_tensor(
                    out=w2b[0:nt, :], in0=qa[0:nt, :], scalar=r1h[0:nt, :],
                    in1=half[0:nt, :], op0=AL.mult, op1=AL.add)
                # alpha = (0.5/tau) * rsqrt(w2b) * r1
                a5r = pw.tile([128, 1], F32, tag="a5r")
                nc.vector.tensor_scalar(out=a5r[0:nt, :], in0=r1[0:nt, :],
                                        scalar1=0.5 / TAU, scalar2=None,
                                        op0=AL.mult)
                alpha = _rsqrt(w2b[0:nt, :], 1, "rw",
                               final_scale_ap=a5r[0:nt, :])

                # --- combine: sc = (qk + winadd) + nq*bl
                tmp = pw.tile([128, 256], F32, tag="tmp")
                nc.vector.tensor_tensor(out=tmp[0:nt, 0:ns],
                                        in0=qke_ps[0:nt, 0:ns],
                                        in1=winadd[0:nt, 0:ns], op=AL.add)
                sc = pw.tile([128, 256], F32, tag="sc")
                nc.vector.scalar_tensor_tensor(
                    out=sc[0:nt, 0:ns], in0=bl_ps[0:nt, 0:ns],
                    scalar=nq[0:nt, :], in1=tmp[0:nt, 0:ns],
                    op0=AL.mult, op1=AL.add)

                # --- softmax with alpha scale
                negmax = pw.tile([128, 1], F32, tag="negmax")
                nc.vector.reduce_max(negmax[0:nt, :], sc[0:nt, 0:ns], axis=X,
                                     negate=True)
                ebias = pw.tile([128, 1], F32, tag="ebias")
                nc.vector.tensor_tensor(out=ebias[0:nt, :], in0=negmax[0:nt, :],
                                        in1=alpha[0:nt, :], op=AL.mult)
                ex = pw.tile([128, 256], F16, tag="ex")
                esum = pw.tile([128, 1], F32, tag="esum")
                nc.scalar.activation(ex[0:nt, 0:ns], sc[0:nt, 0:ns], AF.Exp,
                                     bias=ebias[0:nt, :], scale=alpha[0:nt, :],
                                     accum_out=esum[0:nt, :])
                rsum = pw.tile([128, 1], F32, tag="rsum")
                nc.vector.reciprocal(rsum[0:nt, :], esum[0:nt, :])

                # --- hard match path
                match = pw.tile([128, 256], F32, tag="match")
                msum = pw.tile([128, 1], F32, tag="msum")
                nc.vector.scalar_tensor_tensor(
                    out=match[0:nt, 0:ns], in0=tidb_t[0:nt, 0:ns],
                    scalar=tr_t[0:nt, :], in1=win01[0:nt, 0:ns],
                    op0=AL.is_equal, op1=AL.mult, accum_out=msum[0:nt, :])
                mden = pw.tile([128, 1], F32, tag="mden")
                nc.vector.tensor_scalar(out=mden[0:nt, :], in0=msum[0:nt, :],
                                        scalar1=1e-9, scalar2=None, op0=AL.add)
                mrec = pw.tile([128, 1], F32, tag="mrec")
                nc.vector.reciprocal(mrec[0:nt, :], mden[0:nt, :])
                nohas = pw.tile([128, 1], F32, tag="nohas")
                nc.vector.tensor_scalar(out=nohas[0:nt, :], in0=msum[0:nt, :],
                                        scalar1=0.0, scalar2=None, op0=AL.is_le)
                rs_nh = pw.tile([128, 1], F32, tag="rs_nh")
                nc.vector.tensor_tensor(out=rs_nh[0:nt, :], in0=rsum[0:nt, :],
                                        in1=nohas[0:nt, :], op=AL.mult)
                hard = pw.tile([128, 256], F16, tag="hard")
                nc.vector.tensor_scalar(out=hard[0:nt, 0:ns],
                                        in0=match[0:nt, 0:ns],
                                        scalar1=mrec[0:nt, :], scalar2=None,
                                        op0=AL.mult)
                probs = pw.tile([128, 256], F16, tag="probs")
                nc.vector.scalar_tensor_tensor(
                    out=probs[0:nt, 0:ns], in0=ex[0:nt, 0:ns],
                    scalar=rs_nh[0:nt, :], in1=hard[0:nt, 0:ns],
                    op0=AL.mult, op1=AL.add)

                # --- probs^T (fp16), then val = probs @ V
                pt_ps = pp.tile([128, 264], F16, tag="pt")
                for h in range(nv):
                    nsh = min(128, ns - h * 128)
                    nc.tensor.transpose(pt_ps[0:nsh, h * 128:h * 128 + nt],
                                        probs[0:nt, h * 128:h * 128 + nsh],
                                        ident[0:nt, 0:nt])
                pt16 = pw.tile([128, 2, 128], F16, tag="pt16")
                for h in range(nv):
                    nsh = min(128, ns - h * 128)
                    nc.scalar.copy(pt16[0:nsh, h, 0:nt],
                                   pt_ps[0:nsh, h * 128:h * 128 + nt])
                out16 = pw.tile([128, D], F16, tag="out16")
                for j in range(2):
                    pvj = pp.tile([128, 512], F32, tag=f"pv{j}")
                    for h in range(nv):
                        nsh = min(128, ns - h * 128)
                        nc.tensor.matmul(
                            pvj[0:nt, :],
                            pt16[0:nsh, h, 0:nt],
                            kv_t[0:nsh, voff + h * D + j * 512:
                                 voff + h * D + (j + 1) * 512],
                            start=(h == 0), stop=(h == nv - 1))
                    nc.scalar.copy(out16[0:nt, j * 512:(j + 1) * 512],
                                   pvj[0:nt, :])
                nc.scalar.dma_start(d_out[rows[g]:rows[g] + nt, :],
                                    out16[0:nt, :])
    nc.compile()
    return nc


# ------------------------------------------------------------------ emulator

def _emulate_core(ins, ngs):
    """Numpy emulation of the device kernel, for validation."""
    kv = ins["kv"]
    i16 = sum(_group_geom(ng)[3] for ng in ngs)
    out = np.zeros((i16, D), np.float32)
    winadd, win01, oh8, oh8t16, _ = _consts()
    col = row = 0
    for g, ng in enumerate(ngs):
        ns, nv, ngp, nt, csp, wg = _group_geom(ng)
        ck = kv[:, col:col + KCH * csp].reshape(128, KCH, csp)
        KT = ck[:, :, 0:ns].astype(np.float32)
        AT = ck[:, :, ns:ns + ngp].astype(np.float32)
        QT = ck[:, :, ns + ngp:csp].astype(np.float32)
        voff = col + KCH * csp
        vb = np.zeros((ns, D), np.float32)
        for h in range(nv):
            nsh = min(128, ns - h * 128)
            vb[h * 128:h * 128 + nsh] = \
                kv[0:nsh, voff + h * D:voff + (h + 1) * D].astype(np.float32)
        trp = np.ascontiguousarray(
            kv[0:nt, voff + nv * D:voff + nv * D + 2]).view(np.float32)
        tidb = np.ascontiguousarray(
            kv[0, voff + nv * D + 2:voff + nv * D + 2 + 512]).view(np.float32)

        KTm = KT.transpose(1, 0, 2).reshape(D, ns)
        ATm = AT.transpose(1, 0, 2).reshape(D, ngp)
        QTm = QT.transpose(1, 0, 2).reshape(D, nt)
        qk = QTm.T @ KTm                                # [nt, ns]
        exta = QTm.T @ ATm                              # [nt, ngp]
        ssq = (QTm * QTm).sum(0)[:, None]               # [nt, 1]
        a0t = ATm.T @ KTm                               # [ngp, ns]
        bl = oh8t16[0:ngp, 0:nt].astype(np.float32).T @ a0t
        qa = (exta[:, 0:ng] * oh8[0:nt, 0:ng]).sum(-1, keepdims=True)
        def _qrsqrt(x, n_newton):
            y = (0x5F3759DF - (x.astype(np.float32).view(np.int32) >> 1)) \
                .view(np.float32)
            for _ in range(n_newton):
                y = y * (1.5 - 0.5 * x * y * y)
            return y
        ssqg = ssq + 1e-12
        r1 = _qrsqrt(ssqg, 2)
        nq = ssqg * r1
        w2b = 0.5 * qa * r1 + 0.5
        alpha = (0.5 / TAU) * _qrsqrt(w2b, 1) * r1
        sc = qk + winadd[0:nt, 0:ns] + nq * bl
        m = sc.max(-1, keepdims=True)
        ex = np.exp(alpha * (sc - m))
        esum = ex.sum(-1, keepdims=True)
        match = (tidb[0:ns][None, :] == trp) * win01[0:nt, 0:ns]
        msum = match.sum(-1, keepdims=True)
        nohas = (msum <= 0).astype(np.float32)
        hard = match / (msum + 1e-9)
        probs = ex * (nohas / esum) + hard
        out[row:row + nt] = probs.astype(np.float16).astype(np.float32) @ vb
        col += wg
        row += nt
    return out


# -------------------------------------------------------------------- kernel

def kernel(query_emb, tids, slot_keys, slot_values, slot_tids,
           centroid_codebook, _emulate=False, _trace=False):
    B, T, _ = query_emb.shape
    BT = B * T
    q16 = np.asarray(query_emb, np.float32).reshape(BT, D).astype(np.float16)
    tids_flat = np.asarray(tids).reshape(BT)
    st = np.asarray(slot_tids).astype(np.float32)
    KT16 = np.ascontiguousarray(
        np.asarray(slot_keys, np.float32).T.astype(np.float16))   # [D, S]
    V16 = np.asarray(slot_values, np.float32).astype(np.float16)  # [S, D]
    CBT16 = np.ascontiguousarray(
        np.asarray(centroid_codebook, np.float32).T.astype(np.float16))

    instances = _routing(tids_flat)
    i_core, ngs = _plan(len(instances))
    padded = instances + [None] * (i_core * N_CORES - len(instances))

    winadd, win01, oh8, oh8t16, ident16 = _consts()
    in_maps, tok_idxs = [], []
    for c in range(N_CORES):
        ins, tok_idx = _pack_core(padded[c * i_core:(c + 1) * i_core], ngs,
                                  q16, tids_flat, KT16, V16, st, CBT16)
        ins.update(winadd=winadd, win01=win01, oh8=oh8, oh8t16=oh8t16,
                   ident16=ident16)
        in_maps.append(ins)
        tok_idxs.append(tok_idx)

    out_flat = np.zeros((BT, D), np.float32)
    if _emulate:
        for c in range(N_CORES):
            o = _emulate_core(in_maps[c], ngs)
            valid = tok_idxs[c] >= 0
            out_flat[tok_idxs[c][valid]] = o[valid]
        return out_flat.reshape(B, T, D).astype(np.float32)

    _install_ntff_hook()
    from concourse import bass_utils
    key = ngs
    if key not in _COMPILED:
        _COMPILED[key] = _build_nc(ngs)
    nc = _COMPILED[key]
    res = bass_utils.run_bass_kernel_spmd(
        nc, in_maps, core_ids=list(range(N_CORES)), trace=_trace)
    for c in range(N_CORES):
        o = np.asarray(res.results[c]["outp"], np.float32)
        valid = tok_idxs[c] >= 0
        out_flat[tok_idxs[c][valid]] = o[valid]
    out = out_flat.reshape(B, T, D).astype(np.float32)
    if _trace:
        kernel._last_exec_time_ns = res.exec_time_ns
        kernel._last_results = res
    return out


# revision 13
# speedup vs baseline: 1.4694x; 1.0365x over previous
"""Trainium2 Bass kernel for nn_NexusV2 (CentroidAddressableManifold.read).

Strategy: shard by *bucket* (not token). Tokens are routed host-side to the
core owning their bucket; each bucket's 32 slot rows are loaded exactly once
from HBM (vs. the reference's per-token gather => ~8x less memory traffic).

v2 layout (per core, all shapes static at trace time):
  - tokens are packed into "instances" of <=16 tokens sharing one bucket
  - groups of <=8 instances => <=128 token rows x <=256 slot columns
  - all PE operands packed host-side in fp16 into one kv tile per group:
    per contraction chunk k (8 chunks of 128 dims):
       [ K^T slots (ns) | anchors^T (ngp) | q^T (nt) ]
    then nv V blocks of D cols, then 2 cols holding fp32 token-ids (bitcast).
  - scores = q^T-stationary matmul streaming [K|anchors|q^T]: gives raw q.K,
    q.anchor columns, and the gram block whose diagonal is ||q||^2 -- no
    on-device transposes of q, no activation Square pass.
  - sqrt/rsqrt computed as exp/ln so every ACT op uses one table set
    (natural_log_exp_and_others) => single ACT_TABLE_LOAD for whole kernel.
  - anchor term: a0t = anchors^T.K per group (PE), blended to token rows by a
    {0,1} matmul into a separate PSUM, then combined on DVE as
    sc = blend*2||q|| + q.K which equals (q.K + 2||q|| a.K); one exp scale
    alpha = 0.25/(tau*sqrt(||q||^2 * ||W||^2)) reproduces the reference's
    normalized unified-query scores exactly.
"""

import math
import sys
import types

import numpy as np

N_BUCKETS = 512
SPB = 32          # slots per bucket
TAU = 0.1
P_PAD = 16        # token rows per instance
IPG = 8           # instances per (full) group
N_CORES = 8
D = 1024
KCH = 8           # D / 128 contraction chunks
NEG = -30000.0    # additive mask value
LN2 = math.log(2.0)
BIAS_ALPHA = math.log(0.5 / TAU)

_COMPILED = {}    # plan -> nc
_HOOK_DONE = False


# ----------------------------------------------------------------- utilities

def _install_ntff_hook():
    """Synthesize antenv.axon_hooks so trace=True can NTFF-profile (optional)."""
    global _HOOK_DONE
    if _HOOK_DONE or 'antenv.axon_hooks' in sys.modules:
        _HOOK_DONE = True
        return
    try:
        import antenv
        m = types.ModuleType('antenv.axon_hooks')
        _hook = [None]
        m.set_axon_ntff_profile_hook = lambda h: _hook.__setitem__(0, h)
        m.get_axon_ntff_profile_hook = lambda: _hook[0]
        sys.modules['antenv.axon_hooks'] = m
        antenv.axon_hooks = m
        if '/root/.axon_site' not in sys.path:
            sys.path.insert(0, '/root/.axon_site')
        from trn_agent_boot.trn_boot import _ntff_profile_via_ctypes
        m.set_axon_ntff_profile_hook(
            _ntff_profile_via_ctypes('/opt/axon/libaxon_pjrt.so'))
    except Exception:
        pass
    _HOOK_DONE = True


def _routing(tids_flat):
    """Return list of instances: (bucket_id, np.array of <=16 token indices)."""
    buckets = (tids_flat.astype(np.int64)) % N_BUCKETS
    order = np.argsort(buckets, kind='stable')
    counts = np.bincount(buckets, minlength=N_BUCKETS)
    cum = np.concatenate([[0], np.cumsum(counts)])
    instances = []
    for b in range(N_BUCKETS):
        c = int(counts[b])
        if c == 0:
            continue
        toks = order[cum[b]:cum[b] + c]
        for i in range(0, c, P_PAD):
            instances.append((b, toks[i:i + P_PAD]))
    return instances


def _plan(n_inst):
    i_core = (n_inst + N_CORES - 1) // N_CORES
    ngs, r = [], i_core
    while r > 0:
        ngs.append(min(IPG, r))
        r -= min(IPG, r)
    return i_core, tuple(ngs)


def _group_geom(ng):
    """ns slot cols, nv V blocks, ngp anchors (pad even), nt token rows,
    csp chunk width, wg total kv cols for the group."""
    ns = SPB * ng
    nv = 1 if ns <= 128 else 2
    ngp = ng + (ng % 2)
    nt = P_PAD * ng
    csp = ns + ngp + nt
    wg = KCH * csp + nv * D + 2 + 512
    return ns, nv, ngp, nt, csp, wg


def _consts():
    r = np.arange(128)
    c256 = np.arange(256)
    valid = (c256[None, :] // SPB) == (r[:, None] // P_PAD)
    winadd = np.where(valid, 0.0, NEG).astype(np.float32)
    win01 = valid.astype(np.float32)
    oh8 = (np.arange(IPG)[None, :] == (r[:, None] // P_PAD)).astype(np.float32)
    oh8h = (0.5 * oh8).astype(np.float32)
    oh8t16 = np.ascontiguousarray(oh8.T).astype(np.float16)
    ident16 = np.eye(128, dtype=np.float16)
    # winadd as rank-8 matmul rhs: winadd[t,s] = sum_j oh8[t,j]*maskc[j,s]
    csp_full = _group_geom(IPG)[4]
    maskc = np.zeros((IPG, csp_full), np.float16)
    blk = (c256[None, :] // SPB) == np.arange(IPG)[:, None]
    maskc[:, 0:256] = np.where(blk, 0.0, NEG).astype(np.float16)
    return winadd, win01, oh8h, oh8t16, ident16, maskc


def _pack_core(insts, ngs, q16, tids_flat, KT16, V16, slot_tids, CBT16):
    """Build this core's input arrays. insts: list of (bucket, toks) or None."""
    i16 = sum(_group_geom(ng)[3] for ng in ngs)
    tok_idx = np.full(i16, -1, np.int64)

    wtot = sum(_group_geom(ng)[5] for ng in ngs)
    kv = np.zeros((128, wtot), np.float16)

    col = 0
    row = 0
    ii = 0
    for g, ng in enumerate(ngs):
        ns, nv, ngp, nt, csp, wg = _group_geom(ng)
        group = insts[ii:ii + ng]
        ii += ng
        slot_ids = np.zeros(ns, np.int64)
        real_slots = np.zeros(ns, bool)
        bucks = np.zeros(ng, np.int64)
        real_inst = np.zeros(ng, bool)
        qg = np.zeros((nt, D), np.float16)
        trp = np.full(nt, -1.0, np.float32)
        tidb = np.full(256, -2.0, np.float32)
        for j, inst in enumerate(group):
            if inst is None:
                continue
            b, toks = inst
            bucks[j] = b
            real_inst[j] = True
            slot_ids[j * SPB:(j + 1) * SPB] = np.arange(b * SPB, (b + 1) * SPB)
            real_slots[j * SPB:(j + 1) * SPB] = True
            tidb[j * SPB:(j + 1) * SPB] = slot_tids[b * SPB:(b + 1) * SPB]
            r0 = j * P_PAD
            nt_real = len(toks)
            qg[r0:r0 + nt_real] = q16[toks]
            trp[r0:r0 + nt_real] = tids_flat[toks]
            tok_idx[row + r0:row + r0 + nt_real] = toks
        # chunk block [KCH, 128, csp]: K^T slots | anchors^T | q^T
        ck = np.zeros((KCH, 128, csp), np.float16)
        ck[:, :, 0:ns] = KT16[:, slot_ids].reshape(KCH, 128, ns) \
            * real_slots[None, None, :]
        ck[:, :, ns:ns + ng] = CBT16[:, bucks].reshape(KCH, 128, ng) \
            * real_inst[None, None, :]
        ck[:, :, ns + ngp:csp] = \
            np.ascontiguousarray(qg.T).reshape(KCH, 128, nt)
        kv[:, col:col + KCH * csp] = \
            ck.transpose(1, 0, 2).reshape(128, KCH * csp)
        c = col + KCH * csp
        # V blocks
        vb = V16[slot_ids] * real_slots[:, None]          # [ns, D] fp16
        for h in range(nv):
            nsh = min(128, ns - h * 128)
            kv[0:nsh, c:c + D] = vb[h * 128:h * 128 + nsh]
            c += D
        # fp32 token-ids bitcast into 2 fp16 cols (row t = token t)
        kv[0:nt, c:c + 2] = trp.view(np.float16).reshape(nt, 2)
        # fp32 slot-tid row bitcast into 512 fp16 cols on partition 0
        kv[0, c + 2:c + 2 + 512] = tidb.view(np.float16)
        col += wg
        row += nt
    return dict(kv=kv), tok_idx


# ------------------------------------------------------------- device kernel

def _build_nc(ngs):
    from concourse import bacc, mybir, tile

    F16 = mybir.dt.float16
    F32 = mybir.dt.float32
    I32 = mybir.dt.int32
    AL = mybir.AluOpType
    AF = mybir.ActivationFunctionType
    X = mybir.AxisListType.X

    geoms = [_group_geom(ng) for ng in ngs]
    wtot = sum(g[5] for g in geoms)
    i16 = sum(g[3] for g in geoms)
    n_groups = len(ngs)
    kcols = np.concatenate([[0], np.cumsum([g[5] for g in geoms])])
    rows = np.concatenate([[0], np.cumsum([g[3] for g in geoms])])
    wmax = max(g[5] for g in geoms)

    nc = bacc.Bacc(trn_type="TRN2", target_bir_lowering=False, debug=False)
    d_kv = nc.dram_tensor("kv", [128, wtot], F16, kind="ExternalInput").ap()
    d_winadd = nc.dram_tensor("winadd", [128, 256], F32, kind="ExternalInput").ap()
    d_win01 = nc.dram_tensor("win01", [128, 256], F32, kind="ExternalInput").ap()
    d_oh8 = nc.dram_tensor("oh8", [128, IPG], F32, kind="ExternalInput").ap()
    d_oh8t = nc.dram_tensor("oh8t16", [IPG, 128], F16, kind="ExternalInput").ap()
    d_ident = nc.dram_tensor("ident16", [128, 128], F16, kind="ExternalInput").ap()
    d_maskc = nc.dram_tensor("maskc", [IPG, 392], F16, kind="ExternalInput").ap()
    d_out = nc.dram_tensor("outp", [i16, D], F16, kind="ExternalOutput").ap()

    with tile.TileContext(nc) as tc:
        with tc.tile_pool(name="const", bufs=1) as pc, \
             tc.tile_pool(name="kvp", bufs=4) as pkv, \
             tc.tile_pool(name="io", bufs=3) as pio, \
             tc.tile_pool(name="wk", bufs=2) as pw, \
             tc.tile_pool(name="ps", bufs=1, space="PSUM") as pp:

            winadd = pc.tile([128, 256], F32)
            win01 = pc.tile([128, 256], F32)
            oh8 = pc.tile([128, IPG], F32)
            oh8t = pc.tile([IPG, 128], F16)
            ident = pc.tile([128, 128], F16)
            maskc = pc.tile([IPG, 392], F16)
            nc.scalar.dma_start(winadd[:], d_winadd)
            nc.scalar.dma_start(win01[:], d_win01)
            nc.scalar.dma_start(oh8[:], d_oh8)
            nc.scalar.dma_start(oh8t[:], d_oh8t)
            nc.scalar.dma_start(ident[:], d_ident)
            nc.scalar.dma_start(maskc[:], d_maskc)
            magic = pc.tile([128, 1], I32)
            nc.gpsimd.memset(magic[:], 0x5F3759DF)
            half = pc.tile([128, 1], F32)
            nc.gpsimd.memset(half[:], 0.5)

            for g, ng in enumerate(ngs):
                ns, nv, ngp, nt, csp, wg = geoms[g]
                col = kcols[g]

                kv_t = pkv.tile([128, wmax], F16, tag="kv")
                nc.sync.dma_start(kv_t[:, 0:KCH * csp],
                                  d_kv[:, col:col + KCH * csp])
                nc.sync.dma_start(kv_t[:, KCH * csp:wg],
                                  d_kv[:, col + KCH * csp:col + wg])
                ka = kv_t[:, 0:KCH * csp].rearrange("p (k s) -> p k s", k=KCH)
                voff = KCH * csp
                tr_t = kv_t[:, wg - 514:wg - 512].bitcast(F32)
                tidb_t = pio.tile([128, 256], F32, tag="tidb")
                nc.gpsimd.partition_broadcast(
                    tidb_t[0:nt, 0:ns],
                    kv_t[0:1, wg - 512:wg].bitcast(F32)[:, 0:ns],
                    channels=nt)

                # --- merged scores [q.K | q.a | gram] in one PSUM tile
                qke_ps = pp.tile([128, 392], F32, tag="qke", bufs=2)
                full = ng == IPG
                for k in range(KCH):
                    nc.tensor.matmul(qke_ps[0:nt, 0:csp],
                                     ka[:, k, ns + ngp:csp], ka[:, k, 0:csp],
                                     start=(k == 0),
                                     stop=(k == KCH - 1) and not full)
                if full:
                    nc.tensor.matmul(qke_ps[0:nt, 0:csp], oh8t[0:IPG, 0:nt],
                                     maskc[0:IPG, 0:csp], start=False,
                                     stop=True)

                # --- anchor-dot table a0t = a.K  [ngp, ns]
                a0t_ps = pp.tile([IPG, 256], F32, tag="a0t", bufs=2)
                for k in range(KCH):
                    nc.tensor.matmul(a0t_ps[0:ngp, 0:ns], ka[:, k, ns:ns + ngp],
                                     ka[:, k, 0:ns], start=(k == 0),
                                     stop=(k == KCH - 1))
                a0t16 = pw.tile([IPG, 256], F16, tag="a0t16")
                nc.scalar.copy(a0t16[0:ngp, 0:ns], a0t_ps[0:ngp, 0:ns])

                # --- blend a.K to token rows: bl[t, s] = a0t[inst(t), s]
                bl_ps = pp.tile([128, 256], F32, tag="bl")
                nc.tensor.matmul(bl_ps[0:nt, 0:ns], oh8t[0:ngp, 0:nt],
                                 a0t16[0:ngp, 0:ns], start=True, stop=True)

                # --- per-token scalars from ext columns
                ssq = pw.tile([128, 1], F32, tag="ssq")
                junk = pw.tile([128, 128], F16, tag="junk")
                nc.vector.scalar_tensor_tensor(
                    out=junk[0:nt, 0:nt],
                    in0=qke_ps[0:nt, ns + ngp:ns + ngp + nt],
                    scalar=1.0, in1=ident[0:nt, 0:nt],
                    op0=AL.bypass, op1=AL.mult, accum_out=ssq[0:nt, :])
                qa = pw.tile([128, 1], F32, tag="qa")
                junk8 = pw.tile([128, IPG], F16, tag="junk8")
                nc.vector.scalar_tensor_tensor(
                    out=junk8[0:nt, 0:ng], in0=qke_ps[0:nt, ns:ns + ng],
                    scalar=1.0, in1=oh8[0:nt, 0:ng],
                    op0=AL.bypass, op1=AL.mult, accum_out=qa[0:nt, :])
                # quake rsqrt on DVE: no ACT table switches.
                def _rsqrt(xap, n_newton, tagp, final_scale=None,
                           final_scale_ap=None, tau_fold=False):
                    yt = pw.tile([128, 1], F32, tag=tagp + "y")
                    xs = pw.tile([128, 1], I32, tag=tagp + "xs")
                    nc.vector.tensor_scalar(
                        out=xs[0:nt, :], in0=xap.bitcast(I32), scalar1=1,
                        scalar2=None, op0=AL.logical_shift_right)
                    nc.vector.tensor_tensor(
                        out=yt[0:nt, :].bitcast(I32), in0=magic[0:nt, :],
                        in1=xs[0:nt, :], op=AL.subtract)
                    for it in range(n_newton):
                        t2 = pw.tile([128, 1], F32, tag=f"{tagp}t2{it}")
                        nc.vector.scalar_tensor_tensor(
                            out=t2[0:nt, :], in0=yt[0:nt, :],
                            scalar=xap, in1=yt[0:nt, :],
                            op0=AL.mult, op1=AL.mult)
                        t3 = pw.tile([128, 1], F32, tag=f"{tagp}t3{it}")
                        fld = (0.5 / TAU) if (tau_fold and
                                              it == n_newton - 1) else 1.0
                        nc.vector.tensor_scalar(out=t3[0:nt, :],
                                                in0=t2[0:nt, :],
                                                scalar1=-0.5 * fld,
                                                scalar2=1.5 * fld, op0=AL.mult,
                                                op1=AL.add)
                        yo = pw.tile([128, 1], F32, tag=f"{tagp}y{it}")
                        last = it == n_newton - 1
                        if last and final_scale is not None:
                            nc.vector.tensor_scalar(
                                out=yo[0:nt, :], in0=yt[0:nt, :],
                                scalar1=t3[0:nt, :], scalar2=final_scale,
                                op0=AL.mult, op1=AL.mult)
                        elif last and final_scale_ap is not None:
                            nc.vector.scalar_tensor_tensor(
                                out=yo[0:nt, :], in0=yt[0:nt, :],
                                scalar=t3[0:nt, :], in1=final_scale_ap,
                                op0=AL.mult, op1=AL.mult)
                        else:
                            nc.vector.tensor_tensor(out=yo[0:nt, :],
                                                    in0=yt[0:nt, :],
                                                    in1=t3[0:nt, :],
                                                    op=AL.mult)
                        yt = yo
                    return yt

                ssqg = pw.tile([128, 1], F32, tag="ssqg")
                nc.vector.tensor_scalar(out=ssqg[0:nt, :], in0=ssq[0:nt, :],
                                        scalar1=1e-12, scalar2=None,
                                        op0=AL.add)
                r1 = _rsqrt(ssqg[0:nt, :], 2, "r1")          # 1/sqrt(ssq)
                nq = pw.tile([128, 1], F32, tag="nq")
                nc.vector.tensor_tensor(out=nq[0:nt, :], in0=ssqg[0:nt, :],
                                        in1=r1[0:nt, :], op=AL.mult)
                w2b = pw.tile([128, 1], F32, tag="w2b")
                nc.vector.scalar_tensor_tensor(
                    out=w2b[0:nt, :], in0=qa[0:nt, :], scalar=r1[0:nt, :],
                    in1=half[0:nt, :], op0=AL.mult, op1=AL.add)
                # alpha = (0.5/tau) * rsqrt(w2b) * r1 (0.5/tau baked in t3)
                alpha = _rsqrt(w2b[0:nt, :], 1, "rw",
                               final_scale_ap=r1[0:nt, :], tau_fold=True)

                # --- combine: sc = (qk + winadd) + nq*bl
                bl16 = pw.tile([128, 256], F16, tag="bl16")
                nc.scalar.copy(bl16[0:nt, 0:ns], bl_ps[0:nt, 0:ns])
                if full:
                    qkw = qke_ps[0:nt, 0:ns]
                else:
                    tmp = pw.tile([128, 256], F32, tag="tmp")
                    nc.vector.tensor_tensor(out=tmp[0:nt, 0:ns],
                                            in0=qke_ps[0:nt, 0:ns],
                                            in1=winadd[0:nt, 0:ns], op=AL.add)
                    qkw = tmp[0:nt, 0:ns]
                sc = pw.tile([128, 256], F32, tag="sc")
                nc.vector.scalar_tensor_tensor(
                    out=sc[0:nt, 0:ns], in0=bl16[0:nt, 0:ns],
                    scalar=nq[0:nt, :], in1=qkw,
                    op0=AL.mult, op1=AL.add)

                # --- softmax with alpha scale
                negmax = pw.tile([128, 1], F32, tag="negmax")
                nc.vector.reduce_max(negmax[0:nt, :], sc[0:nt, 0:ns], axis=X,
                                     negate=True)
                ebias = pw.tile([128, 1], F32, tag="ebias")
                nc.vector.tensor_tensor(out=ebias[0:nt, :], in0=negmax[0:nt, :],
                                        in1=alpha[0:nt, :], op=AL.mult)
                ex = pw.tile([128, 256], F16, tag="ex")
                esum = pw.tile([128, 1], F32, tag="esum")
                nc.scalar.activation(ex[0:nt, 0:ns], sc[0:nt, 0:ns], AF.Exp,
                                     bias=ebias[0:nt, :], scale=alpha[0:nt, :],
                                     accum_out=esum[0:nt, :])
                rsum = pw.tile([128, 1], F32, tag="rsum")
                nc.vector.reciprocal(rsum[0:nt, :], esum[0:nt, :])

                # --- hard match path
                match = pw.tile([128, 256], F16, tag="match")
                msum = pw.tile([128, 1], F32, tag="msum")
                nc.vector.scalar_tensor_tensor(
                    out=match[0:nt, 0:ns], in0=tidb_t[0:nt, 0:ns],
                    scalar=tr_t[0:nt, :], in1=win01[0:nt, 0:ns],
                    op0=AL.is_equal, op1=AL.mult, accum_out=msum[0:nt, :])
                mden = pw.tile([128, 1], F32, tag="mden")
                nc.vector.tensor_scalar(out=mden[0:nt, :], in0=msum[0:nt, :],
                                        scalar1=1e-9, scalar2=None, op0=AL.add)
                mrec = pw.tile([128, 1], F32, tag="mrec")
                nc.vector.reciprocal(mrec[0:nt, :], mden[0:nt, :])
                nohas = pw.tile([128, 1], F32, tag="nohas")
                nc.vector.tensor_scalar(out=nohas[0:nt, :], in0=msum[0:nt, :],
                                        scalar1=0.0, scalar2=None, op0=AL.is_le)
                rs_nh = pw.tile([128, 1], F32, tag="rs_nh")
                nc.vector.tensor_tensor(out=rs_nh[0:nt, :], in0=rsum[0:nt, :],
                                        in1=nohas[0:nt, :], op=AL.mult)
                hard = pw.tile([128, 256], F16, tag="hard")
                nc.vector.tensor_scalar(out=hard[0:nt, 0:ns],
                                        in0=match[0:nt, 0:ns],
                                        scalar1=mrec[0:nt, :], scalar2=None,
                                        op0=AL.mult)
                probs = pw.tile([128, 256], F16, tag="probs")
                nc.vector.scalar_tensor_tensor(
                    out=probs[0:nt, 0:ns], in0=ex[0:nt, 0:ns],
                    scalar=rs_nh[0:nt, :], in1=hard[0:nt, 0:ns],
                    op0=AL.mult, op1=AL.add)

                # --- probs^T (fp16), then val = probs @ V
                pt_ps = pp.tile([128, 264], F16, tag="pt")
                for h in range(nv):
                    nsh = min(128, ns - h * 128)
                    nc.tensor.transpose(pt_ps[0:nsh, h * 128:h * 128 + nt],
                                        probs[0:nt, h * 128:h * 128 + nsh],
                                        ident[0:nt, 0:nt])
                pt16 = pw.tile([128, 2, 128], F16, tag="pt16")
                for h in range(nv):
                    nsh = min(128, ns - h * 128)
                    nc.scalar.copy(pt16[0:nsh, h, 0:nt],
                                   pt_ps[0:nsh, h * 128:h * 128 + nt])
                out16 = pw.tile([128, D], F16, tag="out16")
                for j in range(2):
                    pvj = pp.tile([128, 512], F32, tag=f"pv{j}")
                    for h in range(nv):
                        nsh = min(128, ns - h * 128)
                        nc.tensor.matmul(
                            pvj[0:nt, :],
                            pt16[0:nsh, h, 0:nt],
                            kv_t[0:nsh, voff + h * D + j * 512:
                                 voff + h * D + (j + 1) * 512],
                            start=(h == 0), stop=(h == nv - 1))
                    nc.scalar.copy(out16[0:nt, j * 512:(j + 1) * 512],
                                   pvj[0:nt, :])
                nc.scalar.dma_start(d_out[rows[g]:rows[g] + nt, :],
                                    out16[0:nt, :])
    nc.compile()
    return nc


# ------------------------------------------------------------------ emulator

def _emulate_core(ins, ngs):
    """Numpy emulation of the device kernel, for validation."""
    kv = ins["kv"]
    i16 = sum(_group_geom(ng)[3] for ng in ngs)
    out = np.zeros((i16, D), np.float32)
    winadd, win01, oh8h, oh8t16, _, _ = _consts()
    col = row = 0
    for g, ng in enumerate(ngs):
        ns, nv, ngp, nt, csp, wg = _group_geom(ng)
        ck = kv[:, col:col + KCH * csp].reshape(128, KCH, csp)
        KT = ck[:, :, 0:ns].astype(np.float32)
        AT = ck[:, :, ns:ns + ngp].astype(np.float32)
        QT = ck[:, :, ns + ngp:csp].astype(np.float32)
        voff = col + KCH * csp
        vb = np.zeros((ns, D), np.float32)
        for h in range(nv):
            nsh = min(128, ns - h * 128)
            vb[h * 128:h * 128 + nsh] = \
                kv[0:nsh, voff + h * D:voff + (h + 1) * D].astype(np.float32)
        trp = np.ascontiguousarray(
            kv[0:nt, voff + nv * D:voff + nv * D + 2]).view(np.float32)
        tidb = np.ascontiguousarray(
            kv[0, voff + nv * D + 2:voff + nv * D + 2 + 512]).view(np.float32)

        KTm = KT.transpose(1, 0, 2).reshape(D, ns)
        ATm = AT.transpose(1, 0, 2).reshape(D, ngp)
        QTm = QT.transpose(1, 0, 2).reshape(D, nt)
        qk = QTm.T @ KTm                                # [nt, ns]
        exta = QTm.T @ ATm                              # [nt, ngp]
        ssq = (QTm * QTm).sum(0)[:, None]               # [nt, 1]
        a0t = ATm.T @ KTm                               # [ngp, ns]
        bl = oh8t16[0:ngp, 0:nt].astype(np.float32).T @ a0t
        qa = (exta[:, 0:ng] * oh8h[0:nt, 0:ng]).sum(-1, keepdims=True)
        def _qrsqrt(x, n_newton):
            y = (0x5F3759DF - (x.astype(np.float32).view(np.int32) >> 1)) \
                .view(np.float32)
            for _ in range(n_newton):
                y = y * (1.5 - 0.5 * x * y * y)
            return y
        ssqg = ssq + 1e-12
        r1 = _qrsqrt(ssqg, 2)
        nq = ssqg * r1
        w2b = qa * r1 + 0.5
        alpha = (0.5 / TAU) * _qrsqrt(w2b, 1) * r1
        sc = qk + winadd[0:nt, 0:ns] + nq * bl
        m = sc.max(-1, keepdims=True)
        ex = np.exp(alpha * (sc - m))
        esum = ex.sum(-1, keepdims=True)
        match = (tidb[0:ns][None, :] == trp) * win01[0:nt, 0:ns]
        msum = match.sum(-1, keepdims=True)
        nohas = (msum <= 0).astype(np.float32)
        hard = match / (msum + 1e-9)
        probs = ex * (nohas / esum) + hard
        out[row:row + nt] = probs.astype(np.float16).astype(np.float32) @ vb
        col += wg
        row += nt
    return out


# -------------------------------------------------------------------- kernel

def kernel(query_emb, tids, slot_keys, slot_values, slot_tids,
           centroid_codebook, _emulate=False, _trace=False):
    B, T, _ = query_emb.shape
    BT = B * T
    q16 = np.asarray(query_emb, np.float32).reshape(BT, D).astype(np.float16)
    tids_flat = np.asarray(tids).reshape(BT)
    st = np.asarray(slot_tids).astype(np.float32)
    KT16 = np.ascontiguousarray(
        np.asarray(slot_keys, np.float32).T.astype(np.float16))   # [D, S]
    V16 = np.asarray(slot_values, np.float32).astype(np.float16)  # [S, D]
    CBT16 = np.ascontiguousarray(
        np.asarray(centroid_codebook, np.float32).T.astype(np.float16))

    instances = _routing(tids_flat)
    i_core, ngs = _plan(len(instances))
    padded = instances + [None] * (i_core * N_CORES - len(instances))

    winadd, win01, oh8h, oh8t16, ident16, maskc = _consts()
    in_maps, tok_idxs = [], []
    for c in range(N_CORES):
        ins, tok_idx = _pack_core(padded[c * i_core:(c + 1) * i_core], ngs,
                                  q16, tids_flat, KT16, V16, st, CBT16)
        ins.update(winadd=winadd, win01=win01, oh8=oh8h, oh8t16=oh8t16,
                   ident16=ident16, maskc=maskc)
        in_maps.append(ins)
        tok_idxs.append(tok_idx)

    out_flat = np.zeros((BT, D), np.float32)
    if _emulate:
        for c in range(N_CORES):
            o = _emulate_core(in_maps[c], ngs)
            valid = tok_idxs[c] >= 0
            out_flat[tok_idxs[c][valid]] = o[valid]
        return out_flat.reshape(B, T, D).astype(np.float32)

    _install_ntff_hook()
    from concourse import bass_utils
    key = ngs
    if key not in _COMPILED:
        _COMPILED[key] = _build_nc(ngs)
    nc = _COMPILED[key]
    res = bass_utils.run_bass_kernel_spmd(
        nc, in_maps, core_ids=list(range(N_CORES)), trace=_trace)
    for c in range(N_CORES):
        o = np.asarray(res.results[c]["outp"], np.float32)
        valid = tok_idxs[c] >= 0
        out_flat[tok_idxs[c][valid]] = o[valid]
    out = out_flat.reshape(B, T, D).astype(np.float32)
    if _trace:
        kernel._last_exec_time_ns = res.exec_time_ns
        kernel._last_results = res
    return out
